# revision 1
# baseline (speedup 1.0000x reference)
"""AttentionGuidedDynamicRangeDWConv3D on 8 Trainium2 NeuronCores.

Module: out = sum_i softmax(MLP(LN([mean_dhw(x), guidance])))[:, i]
                * dwconv3d(x, convw[i], convb[i], dil=i+1)
Shapes: x [4,96,16,56,56] f32, 3 branches of 3x3x3 depthwise conv with
dilations 1/2/3 ('same' zero padding).

Sharding: 8 cores = (batch b in 0..3) x (depth half h in 0..1). Each core
receives a host-padded 14-plane depth slab (global planes [8h-3, 8h+11),
out-of-range planes zero-filled) so every core runs the identical SPMD
program: owned output planes are always local planes [3, 11).

Layout per core: channels (96) on SBUF partitions, depth*H*W on the free
dim. The 81 conv taps are split between two engines working in parallel:

- VectorE: fused MACs acc = x_shifted * w_eff[c] + acc
  (scalar_tensor_tensor with a per-partition [96,1] weight column).
  'same' padding in H/W via shrunken access patterns.
- TensorE: per-tap diagonal matmuls diag(w_eff[:,t]) @ x_shifted
  accumulated in PSUM, using the 4x-faster fp32r mode (x is
  host-pre-rounded to fp32r's 11-bit mantissa; weights rounded on chip).
  fp32r matmuls require flat contiguous operands, so taps are applied as
  flat shifts over 448-column PSUM chunks; plane-edge chunks are trimmed
  for the h-shift, and the w-shift wraparound columns (which a flat shift
  gets wrong) are fixed up afterwards by small VectorE subtract ops.
  The diagonal weight tiles are rebuilt per (plane, tap) by the otherwise
  idle ScalarE into a 4-slot rotating pool (ScalarE also pre-zeroes the
  PSUM chunks, removing any matmul-accumulation start-flag hazards).

w_eff folds the per-batch softmax gate weights into the per-channel tap
weights. The gate MLP runs redundantly per core on a [1,192] row; the
global pooled features need one cross-core 384-float AllReduce.
"""

import sys

if "/opt/trn_rl_repo" not in sys.path:
    sys.path.insert(0, "/opt/trn_rl_repo")

import numpy as np

import concourse.bass as bass
import concourse.mybir as mybir
import concourse.tile as tile
from concourse.bass_utils import run_bass_kernel_spmd

F32 = mybir.dt.float32
F32R = mybir.dt.float32r
ALU = mybir.AluOpType
ACTF = mybir.ActivationFunctionType

B, C, D, H, W = 4, 96, 16, 56, 56
G, HID, NB = 96, 24, 3
K = 3
DILS = (1, 2, 3)
LN_EPS = 1e-5
N_CORES = 8
DVE_TAPS = 19        # taps computed on DVE
GP_TAPS = 0          # GPSIMD rejected by this walrus (Pool engine check)
CHUNK = 448          # PSUM chunk: 8 h-rows of one plane
N_CHUNKS = 7
XG = 16              # front guard elems for flat-shifted PE reads
XGB = 96             # back guard (fix-up row-slices can overrun the data end)
DH = D // 2          # planes per core (output)
NPL = DH + 2 * 3     # local input planes incl. 3-deep halo/zero pad
HW = H * W
PLANE = HW


def _tap_list():
    """[(tap_col, od, oh, ow)]; center tap of branch 0 first (it initializes
    acc with the bias), then the other ow!=0 taps (DVE side prefers those:
    each PE ow!=0 tap costs an extra wrap fix-up op)."""
    taps = []
    for i, dil in enumerate(DILS):
        for kd in range(K):
            for kh in range(K):
                for kw in range(K):
                    t = i * 27 + kd * 9 + kh * 3 + kw
                    taps.append((t, (kd - 1) * dil, (kh - 1) * dil, (kw - 1) * dil))
    center = 0 * 27 + 1 * 9 + 1 * 3 + 1
    ctr = next(e for e in taps if e[0] == center)
    rest = [e for e in taps if e[0] != center]
    rest.sort(key=lambda e: e[3] == 0)
    return [ctr] + rest


def _build_program():
    nc = bass.Bass()
    xin = nc.dram_tensor("x", [C, NPL * PLANE], F32R, kind="ExternalInput")
    gdin = nc.dram_tensor("gd", [G], F32, kind="ExternalInput")
    cwt_in = nc.dram_tensor("cwt", [C, NB * 27], F32, kind="ExternalInput")
    cbt_in = nc.dram_tensor("cbt", [C, NB], F32, kind="ExternalInput")
    w1t_in = nc.dram_tensor("w1t", [HID, C + G], F32, kind="ExternalInput")
    b1_in = nc.dram_tensor("b1", [HID], F32, kind="ExternalInput")
    w2_in = nc.dram_tensor("w2", [HID, NB], F32, kind="ExternalInput")
    b2_in = nc.dram_tensor("b2", [NB], F32, kind="ExternalInput")
    lng_in = nc.dram_tensor("lng", [C + G], F32, kind="ExternalInput")
    lnb_in = nc.dram_tensor("lnb", [C + G], F32, kind="ExternalInput")
    oh4_in = nc.dram_tensor("oh4", [C, B], F32, kind="ExternalInput")
    id_in = nc.dram_tensor("idp", [C, C], F32, kind="ExternalInput")
    yout = nc.dram_tensor("y", [C, DH * PLANE], F32, kind="ExternalOutput")

    with tile.TileContext(nc) as tc:
        with (
            tc.tile_pool(name="sbuf", bufs=1) as pool,
            tc.tile_pool(name="diagp", bufs=4) as diagpool,
            tc.tile_pool(name="dram", bufs=1, space="DRAM") as dpool,
            tc.tile_pool(name="psum", bufs=1, space="PSUM") as ppool,
        ):
            xbuf = pool.tile([C, XG + NPL * PLANE + XGB], F32R, tag="xbuf")
            acc = pool.tile([C, PLANE], F32, tag="acc")
            acc2 = pool.tile([C, PLANE], F32, tag="acc2") if GP_TAPS else None
            w_eff = pool.tile([C, NB * 27], F32, tag="w_eff")
            w_neg = pool.tile([C, NB * 27], F32, tag="w_neg")
            cwt = pool.tile([C, NB * 27], F32, tag="cwt")
            cbt = pool.tile([C, NB], F32, tag="cbt")
            b_eff = pool.tile([C, 1], F32, tag="b_eff")
            tmpb = pool.tile([C, NB], F32, tag="tmpb")
            onehot_bc = pool.tile([C, B], F32, tag="onehot_bc")
            featp = pool.tile([C, 1], F32, tag="featp")
            contrib = pool.tile([C, B], F32, tag="contrib")
            ar_s = pool.tile([C, B], F32, tag="ar_s")
            feat_full = pool.tile([C, 1], F32, tag="feat_full")
            g_row = pool.tile([1, C + G], F32, tag="g_row")
            gd_row = pool.tile([1, C + G], F32, tag="gd_row")
            lng = pool.tile([1, C + G], F32, tag="lng")
            lnb = pool.tile([1, C + G], F32, tag="lnb")
            gn_row = pool.tile([1, C + G], F32, tag="gn_row")
            gn_bc = pool.tile([HID, C + G], F32, tag="gn_bc")
            w1t = pool.tile([HID, C + G], F32, tag="w1t")
            prod = pool.tile([HID, C + G], F32, tag="prod")
            hvec = pool.tile([HID, 1], F32, tag="hvec")
            b1c = pool.tile([HID, 1], F32, tag="b1c")
            w2t = pool.tile([HID, NB], F32, tag="w2t")
            l2tmp = pool.tile([HID, NB], F32, tag="l2tmp")
            z72 = pool.tile([1, HID * NB], F32, tag="z72")
            zrow = pool.tile([1, NB], F32, tag="zrow")
            b2r = pool.tile([1, NB], F32, tag="b2r")
            wts = pool.tile([1, NB], F32, tag="wts")
            wts_bc = pool.tile([C, NB], F32, tag="wts_bc")
            idp = pool.tile([C, C], F32, tag="idp")
            s1 = pool.tile([1, 1], F32, tag="s1")
            s2 = pool.tile([1, 1], F32, tag="s2")
            s3 = pool.tile([1, 1], F32, tag="s3")
            s4 = pool.tile([1, 1], F32, tag="s4")

            cin = dpool.tile([C, B], F32, tag="cin")
            cout = dpool.tile([C, B], F32, tag="cout")
            fb = dpool.tile([1, C], F32, tag="fb")
            zt = dpool.tile([1, HID * NB], F32, tag="zt")
            gb = dpool.tile([1, C + G], F32, tag="gb")
            wb = dpool.tile([1, NB], F32, tag="wb")

            v = nc.vector
            sc = nc.scalar

            # ---- loads ----
            o0, o1 = 3 * PLANE, (3 + DH) * PLANE
            mid = 3 * PLANE + (DH // 2) * PLANE
            nc.sync.dma_start(out=xbuf[:, XG + o0 : XG + mid], in_=xin[:, o0:mid])
            nc.sync.dma_start(out=xbuf[:, XG + mid : XG + o1], in_=xin[:, mid:o1])
            nc.sync.dma_start(out=xbuf[:, XG : XG + o0], in_=xin[:, :o0])
            nc.sync.dma_start(out=xbuf[:, XG + o1 : XG + NPL * PLANE], in_=xin[:, o1:])
            nc.sync.dma_start(out=cwt[:, :], in_=cwt_in[:, :])
            nc.sync.dma_start(out=cbt[:, :], in_=cbt_in[:, :])
            nc.sync.dma_start(out=w1t[:, :], in_=w1t_in[:, :])
            nc.sync.dma_start(out=b1c[:, :], in_=b1_in[:, None])
            nc.sync.dma_start(out=w2t[:, :], in_=w2_in[:, :])
            nc.sync.dma_start(out=b2r[:, :], in_=b2_in[None, :])
            nc.sync.dma_start(out=lng[:, :], in_=lng_in[None, :])
            nc.sync.dma_start(out=lnb[:, :], in_=lnb_in[None, :])
            nc.sync.dma_start(out=onehot_bc[:, :], in_=oh4_in[:, :])
            nc.sync.dma_start(out=idp[:, :], in_=id_in[:, :])
            nc.sync.dma_start(out=g_row[:, C:], in_=gdin[None, :])

            xflat_r = xbuf[:, :]                       # fp32r view (PE rhs)
            xflat = xbuf[:, :].bitcast(F32)            # f32 view (DVE)
            xv = xflat[:, XG : XG + NPL * PLANE].rearrange(
                "c (d h w) -> c d h w", d=NPL, h=H, w=W
            )

            # ---- global-pool partial over owned planes [3, 3+DH) ----
            v.reduce_sum(featp[:, :], xv[:, 3 : 3 + DH // 2], axis=mybir.AxisListType.XYZ)
            v.reduce_sum(tmpb[:, 0:1], xv[:, 3 + DH // 2 : 3 + DH], axis=mybir.AxisListType.XYZ)
            v.tensor_tensor(out=featp[:, :], in0=featp[:, :], in1=tmpb[:, 0:1], op=ALU.add)
            v.tensor_scalar_mul(featp[:, :], featp[:, :], 1.0 / (D * HW))
            v.tensor_scalar(
                out=contrib[:, :], in0=onehot_bc[:, :], scalar1=featp[:, :],
                scalar2=None, op0=ALU.mult,
            )

            # ---- cross-core AllReduce of [C, B] partials ----
            nc.sync.dma_start(out=cin[:, :], in_=contrib[:, :])
            nc.gpsimd.collective_compute(
                "AllReduce",
                ALU.add,
                replica_groups=[list(range(N_CORES))],
                ins=[cin.opt()],
                outs=[cout.opt()],
            )
            nc.sync.dma_start(out=ar_s[:, :], in_=cout[:, :])
            v.tensor_tensor(out=ar_s[:, :], in0=ar_s[:, :], in1=onehot_bc[:, :], op=ALU.mult)
            v.reduce_sum(feat_full[:, :], ar_s[:, :], axis=mybir.AxisListType.X)

            # ---- bounce feat to a single-partition row, build g=[feat|guidance]
            nc.sync.dma_start(out=fb[:, :], in_=feat_full[:, :])
            nc.sync.dma_start(out=g_row[:, :C], in_=fb[:, :])

            # ---- LayerNorm over 192 on one partition ----
            v.reduce_sum(s1[:, :], g_row[:, :], axis=mybir.AxisListType.X)
            v.tensor_scalar_mul(s1[:, :], s1[:, :], 1.0 / (C + G))  # mu
            v.tensor_scalar(
                out=gd_row[:, :], in0=g_row[:, :], scalar1=s1[:, :], scalar2=None,
                op0=ALU.subtract,
            )
            v.tensor_tensor(out=gn_row[:, :], in0=gd_row[:, :], in1=gd_row[:, :], op=ALU.mult)
            v.reduce_sum(s2[:, :], gn_row[:, :], axis=mybir.AxisListType.X)
            v.tensor_scalar(
                out=s2[:, :], in0=s2[:, :], scalar1=1.0 / (C + G), scalar2=LN_EPS,
                op0=ALU.mult, op1=ALU.add,
            )  # var + eps
            sc.activation(s3[:, :], s2[:, :], ACTF.Sqrt)
            # one Newton step: s4 = 0.5*(s3 + (var+eps)/s3) for a clean sqrt
            v.reciprocal(s4[:, :], s3[:, :])
            v.tensor_tensor(out=s4[:, :], in0=s4[:, :], in1=s2[:, :], op=ALU.mult)
            v.tensor_tensor(out=s4[:, :], in0=s4[:, :], in1=s3[:, :], op=ALU.add)
            v.tensor_scalar_mul(s4[:, :], s4[:, :], 0.5)
            v.reciprocal(s3[:, :], s4[:, :])  # rstd
            v.tensor_scalar(
                out=gn_row[:, :], in0=gd_row[:, :], scalar1=s3[:, :], scalar2=None,
                op0=ALU.mult,
            )
            v.tensor_tensor(out=gn_row[:, :], in0=gn_row[:, :], in1=lng[:, :], op=ALU.mult)
            v.tensor_tensor(out=gn_row[:, :], in0=gn_row[:, :], in1=lnb[:, :], op=ALU.add)

            # ---- MLP layer 1: h = gelu(gn @ w1 + b1) via row-products ----
            nc.sync.dma_start(out=gb[:, :], in_=gn_row[:, :])
            nc.sync.dma_start(out=gn_bc[:, :], in_=gb[:1, :].partition_broadcast(HID))
            v.tensor_tensor(out=prod[:, :], in0=w1t[:, :], in1=gn_bc[:, :], op=ALU.mult)
            v.reduce_sum(hvec[:, :], prod[:, :], axis=mybir.AxisListType.X)
            v.tensor_tensor(out=hvec[:, :], in0=hvec[:, :], in1=b1c[:, :], op=ALU.add)
            sc.activation(hvec[:, :], hvec[:, :], ACTF.Gelu)

            # ---- MLP layer 2 via DRAM transpose bounce ----
            v.tensor_scalar(
                out=l2tmp[:, :], in0=w2t[:, :], scalar1=hvec[:, :], scalar2=None,
                op0=ALU.mult,
            )
            nc.sync.dma_start(out=zt[:, :], in_=l2tmp[:, :])
            nc.sync.dma_start(out=z72[:, :], in_=zt[:, :])
            z3 = z72[:, :].rearrange("a (j i) -> a j i", j=HID, i=NB)
            for i in range(NB):
                v.reduce_sum(zrow[:, i : i + 1], z3[:, :, i], axis=mybir.AxisListType.X)
            v.tensor_tensor(out=zrow[:, :], in0=zrow[:, :], in1=b2r[:, :], op=ALU.add)

            # ---- softmax over 3 ----
            v.reduce_max(s1[:, :], zrow[:, :], axis=mybir.AxisListType.X)
            v.tensor_scalar(
                out=zrow[:, :], in0=zrow[:, :], scalar1=s1[:, :], scalar2=None,
                op0=ALU.subtract,
            )
            sc.activation(zrow[:, :], zrow[:, :], ACTF.Exp)
            v.reduce_sum(s2[:, :], zrow[:, :], axis=mybir.AxisListType.X)
            v.reciprocal(s2[:, :], s2[:, :])
            v.tensor_scalar(
                out=wts[:, :], in0=zrow[:, :], scalar1=s2[:, :], scalar2=None,
                op0=ALU.mult,
            )

            # ---- fold gate weights into per-tap channel weights ----
            nc.sync.dma_start(out=wb[:, :], in_=wts[:, :])
            nc.sync.dma_start(out=wts_bc[:, :], in_=wb[:1, :].partition_broadcast(C))
            for i in range(NB):
                v.tensor_scalar(
                    out=w_eff[:, i * 27 : (i + 1) * 27],
                    in0=cwt[:, i * 27 : (i + 1) * 27],
                    scalar1=wts_bc[:, i : i + 1],
                    scalar2=None,
                    op0=ALU.mult,
                )
            v.tensor_scalar_mul(w_neg[:, :], w_eff[:, :], -1.0)
            v.tensor_tensor(out=tmpb[:, :], in0=cbt[:, :], in1=wts_bc[:, :], op=ALU.mult)
            v.reduce_sum(b_eff[:, :], tmpb[:, :], axis=mybir.AxisListType.X)

            # ---- the conv ----
            taps = _tap_list()
            dve_taps = taps[:DVE_TAPS]
            gp_taps = taps[DVE_TAPS : DVE_TAPS + GP_TAPS]
            pe_taps = taps[DVE_TAPS + GP_TAPS :]
            # a full-extent tap must run first per chunk: its start=True
            # matmul claims every PSUM cell, so no memzero pass is needed
            pe_taps.sort(key=lambda e: not (e[2] == 0 and e[3] == 0))
            assert pe_taps[0][2] == 0 and pe_taps[0][3] == 0
            accv = acc[:, :].rearrange("c (h w) -> c h w", h=H, w=W)
            for p in range(3, 3 + DH):
                # PE side: ScalarE zeroes psum chunks and rebuilds each tap's
                # diagonal; TensorE runs flat fp32r matmuls per 448-col chunk.
                pss = []
                for ci in range(N_CHUNKS):
                    ps = ppool.tile([C, CHUNK], F32,
                                    tag=f"ps{((p - 3) * N_CHUNKS + ci) % 8}")
                    pss.append(ps)
                for tn, (t, od, oh, ow) in enumerate(pe_taps):
                    dg = diagpool.tile([C, C], F32R, tag="diag")
                    sc.activation(dg[:, :], idp[:, :], ACTF.Copy,
                                  scale=w_eff[:, t : t + 1])
                    lo_trim = max(0, -oh) * W
                    hi_trim = (H - max(0, oh)) * W
                    for ci in range(N_CHUNKS):
                        a = max(ci * CHUNK, lo_trim)
                        b = min((ci + 1) * CHUNK, hi_trim)
                        src0 = XG + (p + od) * PLANE + a + oh * W + ow
                        nc.tensor.matmul(
                            pss[ci][:, a - ci * CHUNK : b - ci * CHUNK],
                            dg[:, :],
                            xflat_r[:, src0 : src0 + (b - a)],
                            start=(tn == 0),
                            stop=False,
                            skip_group_check=True,
                        )
                # DVE side: exact fp32 MACs (on the rounded x)
                for n, (t, od, oh, ow) in enumerate(dve_taps):
                    h0i, h1i = max(0, oh), H + min(0, oh)
                    w0i, w1i = max(0, ow), W + min(0, ow)
                    h0o, h1o = max(0, -oh), H + min(0, -oh)
                    w0o, w1o = max(0, -ow), W + min(0, -ow)
                    in_ap = xv[:, p + od, h0i:h1i, w0i:w1i]
                    out_ap = accv[:, h0o:h1o, w0o:w1o]
                    if n == 0:
                        # full-extent center tap initializes acc with bias
                        v.tensor_scalar(
                            out=out_ap, in0=in_ap, scalar1=w_eff[:, t : t + 1],
                            scalar2=b_eff[:, :], op0=ALU.mult, op1=ALU.add,
                        )
                    else:
                        v.scalar_tensor_tensor(
                            out=out_ap, in0=in_ap, scalar=w_eff[:, t : t + 1],
                            in1=out_ap, op0=ALU.mult, op1=ALU.add,
                        )
                # GPSIMD side: extra taps into a separate accumulator
                for n, (t, od, oh, ow) in enumerate(gp_taps):
                    h0i, h1i = max(0, oh), H + min(0, oh)
                    w0i, w1i = max(0, ow), W + min(0, ow)
                    h0o, h1o = max(0, -oh), H + min(0, -oh)
                    w0o, w1o = max(0, -ow), W + min(0, -ow)
                    in_ap = xv[:, p + od, h0i:h1i, w0i:w1i]
                    out_ap = acc2[:, :].rearrange(
                        "c (h w) -> c h w", h=H, w=W
                    )[:, h0o:h1o, w0o:w1o]
                    if n == 0:
                        nc.gpsimd.tensor_scalar(
                            out=acc2[:, :], in0=acc2[:, :], scalar1=0.0,
                            scalar2=None, op0=ALU.mult,
                        )
                        nc.gpsimd.scalar_tensor_tensor(
                            out=out_ap, in0=in_ap, scalar=w_eff[:, t : t + 1],
                            in1=out_ap, op0=ALU.mult, op1=ALU.add,
                        )
                    else:
                        nc.gpsimd.scalar_tensor_tensor(
                            out=out_ap, in0=in_ap, scalar=w_eff[:, t : t + 1],
                            in1=out_ap, op0=ALU.mult, op1=ALU.add,
                        )
                if gp_taps:
                    v.tensor_tensor(out=acc[:, :], in0=acc[:, :], in1=acc2[:, :],
                                    op=ALU.add)
                # merge PSUM chunks into acc
                for ci in range(N_CHUNKS):
                    seg = slice(ci * CHUNK, (ci + 1) * CHUNK)
                    v.tensor_tensor(
                        out=acc[:, seg], in0=acc[:, seg], in1=pss[ci][:, :],
                        op=ALU.add,
                    )
                # fix up the w-wrap columns the flat PE shifts got wrong
                for t, od, oh, ow in pe_taps:
                    if ow == 0:
                        continue
                    r0 = max(0, -oh)
                    nr = H - abs(oh)
                    w0 = W - ow if ow > 0 else 0
                    nw = abs(ow)
                    base = XG + (p + od) * PLANE + (r0 + oh) * W + (w0 + ow)
                    src = xflat[:, base : base + nr * W].rearrange(
                        "c (r w) -> c r w", r=nr, w=W
                    )[:, :, 0:nw]
                    out2d = accv[:, r0 : r0 + nr, w0 : w0 + nw]
                    v.scalar_tensor_tensor(
                        out=out2d, in0=src, scalar=w_neg[:, t : t + 1],
                        in1=out2d, op0=ALU.mult, op1=ALU.add,
                    )
                nc.sync.dma_start(
                    out=yout[:, (p - 3) * PLANE : (p - 2) * PLANE], in_=acc[:, :]
                )

    _split_sem_waits(nc)
    return nc


_WAITSPLIT = [0]


def _split_sem_waits(nc, max_waits=1):
    """This walrus build rejects >1 SyncWait per instruction (and any wait on
    a Drain). Move excess waits onto same-engine NOPs inserted just before."""
    for bb in nc.main_func.blocks:
        insns = bb.instructions
        i = 0
        while i < len(insns):
            ins = insns[i]
            si = ins.sync_info
            limit = 0 if ins.opcode == "Drain" else max_waits
            if si is not None and si.on_wait is not None and len(si.on_wait) > limit:
                waits = list(si.on_wait)
                keep = waits[-limit:] if limit else []
                extra = waits[: len(waits) - limit]
                pos = i
                for j in range(0, len(extra), max_waits):
                    nop = mybir.InstNoOp(
                        name=f"I-waitsplit-{_WAITSPLIT[0]}", ins=[], outs=[]
                    )
                    _WAITSPLIT[0] += 1
                    nop.engine = ins.engine
                    nop.sync_info = mybir.SyncInfo(
                        on_wait=extra[j : j + max_waits], on_update=[]
                    )
                    insns.insert(pos, nop)
                    pos += 1
                    i += 1
                si.on_wait = keep
            i += 1


def _round_fp32r(a):
    u = np.ascontiguousarray(a, dtype=np.float32).view(np.uint32)
    lsb = (u >> 12) & 1
    r = ((u + 0x7FF + lsb) & np.uint32(0xFFFFF000)).astype(np.uint32)
    return r.view(np.float32)


def _prep_inputs(x, guidance, convw, convb, ln_g, ln_b, w1, b1, w2, b2):
    f = np.float32
    cwt = np.ascontiguousarray(
        convw.reshape(NB, C, 27).transpose(1, 0, 2).reshape(C, NB * 27), dtype=f
    )
    cbt = np.ascontiguousarray(convb.T, dtype=f)
    w1t = np.ascontiguousarray(w1.T, dtype=f)
    idp = np.eye(C, dtype=f)
    common = dict(
        cwt=cwt, cbt=cbt, w1t=w1t,
        b1=np.ascontiguousarray(b1, dtype=f),
        w2=np.ascontiguousarray(w2, dtype=f),
        b2=np.ascontiguousarray(b2, dtype=f),
        lng=np.ascontiguousarray(ln_g, dtype=f),
        lnb=np.ascontiguousarray(ln_b, dtype=f),
        idp=idp,
    )
    in_maps = []
    for core in range(N_CORES):
        b, h = core // 2, core % 2
        lo = 8 * h - 3
        shard = np.zeros((C, NPL, H, W), dtype=f)
        g0, g1 = max(0, lo), min(D, lo + NPL)
        shard[:, g0 - lo : g1 - lo] = x[b, :, g0:g1]
        onehot = np.zeros((C, B), dtype=f)
        onehot[:, b] = 1.0
        in_maps.append(
            dict(
                x=_round_fp32r(shard.reshape(C, NPL * PLANE)),
                gd=np.ascontiguousarray(guidance[b], dtype=f),
                oh4=onehot,
                **common,
            )
        )
    return in_maps


_CACHED_NC = None


def kernel(x, guidance, convw, convb, ln_g, ln_b, w1, b1, w2, b2):
    global _CACHED_NC
    if _CACHED_NC is None:
        _CACHED_NC = _build_program()
    in_maps = _prep_inputs(
        x, guidance, convw, convb, ln_g, ln_b, w1, b1, w2, b2
    )
    res = run_bass_kernel_spmd(_CACHED_NC, in_maps, list(range(N_CORES)))
    out = np.empty((B, C, D, H, W), dtype=np.float32)
    for core in range(N_CORES):
        b, h = core // 2, core % 2
        out[b, :, 8 * h : 8 * h + 8] = res.results[core]["y"].reshape(C, DH, H, W)
    return out


if __name__ == "__main__":
    rng = np.random.default_rng(0)
    ins = dict(
        x=rng.standard_normal((B, C, D, H, W), dtype=np.float32),
        guidance=rng.standard_normal((B, G), dtype=np.float32),
        convw=(rng.standard_normal((NB, C, 1, K, K, K)) * 0.1).astype(np.float32),
        convb=np.zeros((NB, C), np.float32),
        ln_g=np.ones((C + G,), np.float32),
        ln_b=np.zeros((C + G,), np.float32),
        w1=(rng.standard_normal((C + G, HID)) * 0.05).astype(np.float32),
        b1=np.zeros((HID,), np.float32),
        w2=(rng.standard_normal((HID, NB)) * 0.05).astype(np.float32),
        b2=np.zeros((NB,), np.float32),
    )
    out = kernel(**ins)
    print("kernel ran, out shape", out.shape, "mean", float(np.abs(out).mean()))



# revision 8
# speedup vs baseline: 2.9073x; 2.9073x over previous
"""AttentionGuidedDynamicRangeDWConv3D on 8 Trainium2 NeuronCores.

Module: out = sum_i softmax(MLP(LN([mean_dhw(x), guidance])))[:, i]
                * dwconv3d(x, convw[i], convb[i], dil=i+1)
Shapes: x [4,96,16,56,56] f32, 3 branches of 3x3x3 depthwise conv with
dilations 1/2/3 ('same' zero padding).

Sharding: 8 cores = (batch b in 0..3) x (channel half hc in 0..1); each
core owns 48 channels of one batch at FULL depth.

Layout trick: partitions = (channel c in 0..8) x (depth d in 0..16), so a
single bf16 matmul with a 128x128 block-banded weight matrix applies an
entire depth-band of conv taps at once: out[(c,d), hw] +=
sum_od w[c, (od,oh,ow)] * x[(c,d+od), hw + oh*56+ow].  The 81 taps
(3 branches x 27) collapse into 25 matmul passes -- one per distinct
(oh,ow) pair -- accumulated in PSUM per 448-column chunk.  Depth 'same'
padding falls out of band truncation (no halo).  H/W 'same' padding is
exact via trimmed 2D access patterns (rows h with h+oh OOB and cols w
with w+ow OOB are simply excluded; bf16 matmuls allow strided APs).

The band matrices are built by the Vector engine from host-supplied
shifted-identity masks scaled by per-partition weight columns
(w_exp = convw x softmax gate weight, folded on-chip after the MLP).
The gate MLP runs redundantly per core; the global mean-pool needs one
96-float AllReduce across the 2 cores of each batch (pair groups).
PSUM results are copied to SBUF by the Scalar engine and DMAd out.
"""

import sys

if "/opt/trn_rl_repo" not in sys.path:
    sys.path.insert(0, "/opt/trn_rl_repo")

import ml_dtypes
import numpy as np

import concourse.bass as bass
import concourse.mybir as mybir
import concourse.tile as tile
from concourse.bass_utils import run_bass_kernel_spmd

F32 = mybir.dt.float32
BF16 = mybir.dt.bfloat16
ALU = mybir.AluOpType
ACTF = mybir.ActivationFunctionType

B, C, D, H, W = 4, 96, 16, 56, 56
G, HID, NB = 96, 24, 3
K = 3
DILS = (1, 2, 3)
LN_EPS = 1e-5
N_CORES = 8
NCH = 8                  # channels per slab
NSLAB = 6                # slabs per core (48 channels)
NP = NCH * D             # 128 partitions
HW = H * W               # 3136
CHUNK = 448              # 8 h-rows; PSUM bank-sized chunk
N_CHUNKS = 7
ROWS = CHUNK // W        # 8


def _pass_list():
    """[(oh, ow, [(od, t), ...])]; (0,0) first (its matmul claims the full
    chunk with start=True)."""
    pairs = [(0, 0)]
    for d in DILS:
        for oh in (-d, 0, d):
            for ow in (-d, 0, d):
                if (oh, ow) != (0, 0) and (oh, ow) not in pairs:
                    pairs.append((oh, ow))
    passes = []
    for oh, ow in pairs:
        entries = []
        for i, d in enumerate(DILS):
            if oh in (-d, 0, d) and ow in (-d, 0, d):
                kh = oh // d + 1
                kw = ow // d + 1
                for kd in range(K):
                    od = (kd - 1) * d
                    t = i * 27 + kd * 9 + kh * 3 + kw
                    entries.append((od, t))
        passes.append((oh, ow, entries))
    assert len(passes) == 25
    assert sum(len(e) for _, _, e in passes) == 81
    return passes


def _build_program(with_bias):
    nc = bass.Bass()
    xin = nc.dram_tensor("x", [NP, NSLAB * HW], BF16, kind="ExternalInput")
    masks_in = nc.dram_tensor("masks", [NP, 7 * NP], BF16, kind="ExternalInput")
    cwx_in = nc.dram_tensor("cwx", [NP, NSLAB * 27 * NB], F32, kind="ExternalInput")
    cbx_in = nc.dram_tensor("cbx", [NP, NSLAB * NB], F32, kind="ExternalInput")
    csel_in = nc.dram_tensor("csel", [NP, NSLAB * 48], F32, kind="ExternalInput")
    zr_in = nc.dram_tensor("zr48", [48, 1], F32, kind="ExternalInput")
    gdin = nc.dram_tensor("gd", [G], F32, kind="ExternalInput")
    w1t_in = nc.dram_tensor("w1t", [HID, C + G], F32, kind="ExternalInput")
    b1_in = nc.dram_tensor("b1", [HID], F32, kind="ExternalInput")
    w2_in = nc.dram_tensor("w2", [HID, NB], F32, kind="ExternalInput")
    b2_in = nc.dram_tensor("b2", [NB], F32, kind="ExternalInput")
    lng_in = nc.dram_tensor("lng", [C + G], F32, kind="ExternalInput")
    lnb_in = nc.dram_tensor("lnb", [C + G], F32, kind="ExternalInput")
    hcsel_in = nc.dram_tensor("hcsel", [1, 2], F32, kind="ExternalInput")
    yout = nc.dram_tensor("y", [NP, NSLAB * HW], F32, kind="ExternalOutput")

    passes = _pass_list()

    with tile.TileContext(nc) as tc:
        with (
            tc.tile_pool(name="sbuf", bufs=1) as pool,
            tc.tile_pool(name="mats", bufs=2) as matpool,
            tc.tile_pool(name="outs", bufs=4) as outpool,
            tc.tile_pool(name="dram", bufs=1, space="DRAM") as dpool,
            tc.tile_pool(name="psum", bufs=1, space="PSUM") as ppool,
        ):
            xs = pool.tile([NP, NSLAB * HW], BF16, tag="xs")
            masks = pool.tile([NP, 7 * NP], BF16, tag="masks")
            cwx = pool.tile([NP, NSLAB * 27 * NB], F32, tag="cwx")
            w_exp = pool.tile([NP, NSLAB * 27 * NB], F32, tag="w_exp")
            csel = pool.tile([NP, NSLAB * 48], F32, tag="csel")
            part = pool.tile([NP, NSLAB], F32, tag="part")
            featp = pool.tile([48, 1], F32, tag="featp")
            g_row = pool.tile([1, C + G], F32, tag="g_row")
            gd_row = pool.tile([1, C + G], F32, tag="gd_row")
            lng = pool.tile([1, C + G], F32, tag="lng")
            lnb = pool.tile([1, C + G], F32, tag="lnb")
            gn_row = pool.tile([1, C + G], F32, tag="gn_row")
            gn_bc = pool.tile([HID, C + G], F32, tag="gn_bc")
            w1t = pool.tile([HID, C + G], F32, tag="w1t")
            prod = pool.tile([HID, C + G], F32, tag="prod")
            hvec = pool.tile([HID, 1], F32, tag="hvec")
            b1c = pool.tile([HID, 1], F32, tag="b1c")
            w2t = pool.tile([HID, NB], F32, tag="w2t")
            l2tmp = pool.tile([HID, NB], F32, tag="l2tmp")
            z72 = pool.tile([1, HID * NB], F32, tag="z72")
            zrow = pool.tile([1, NB], F32, tag="zrow")
            b2r = pool.tile([1, NB], F32, tag="b2r")
            wts = pool.tile([1, NB], F32, tag="wts")
            wts_bc = pool.tile([NP, NB], F32, tag="wts_bc")
            s1 = pool.tile([1, 1], F32, tag="s1")
            s2 = pool.tile([1, 1], F32, tag="s2")
            s3 = pool.tile([1, 1], F32, tag="s3")
            s4 = pool.tile([1, 1], F32, tag="s4")
            if with_bias:
                cbx = pool.tile([NP, NSLAB * NB], F32, tag="cbx")
                b_exp = pool.tile([NP, NSLAB], F32, tag="b_exp")
                betmp = pool.tile([NP, NSLAB * NB], F32, tag="betmp")

            cin = dpool.tile([48, 1], F32, tag="cin")
            cout = dpool.tile([C, 1], F32, tag="cout")
            zt = dpool.tile([1, HID * NB], F32, tag="zt")
            gb = dpool.tile([1, C + G], F32, tag="gb")
            wb = dpool.tile([1, NB], F32, tag="wb")

            v = nc.vector
            sc = nc.scalar

            # ---- loads: x slab-by-slab so pool reduces pipeline ----
            for s in range(NSLAB):
                nc.sync.dma_start(
                    out=xs[:, s * HW : (s + 1) * HW],
                    in_=xin[:, s * HW : (s + 1) * HW],
                )
            nc.sync.dma_start(out=masks[:, :], in_=masks_in[:, :])
            nc.sync.dma_start(out=cwx[:, :], in_=cwx_in[:, :])
            nc.sync.dma_start(out=csel[:, :], in_=csel_in[:, :])
            nc.sync.dma_start(out=w1t[:, :], in_=w1t_in[:, :])
            nc.sync.dma_start(out=b1c[:, :], in_=b1_in[:, None])
            nc.sync.dma_start(out=w2t[:, :], in_=w2_in[:, :])
            nc.sync.dma_start(out=b2r[:, :], in_=b2_in[None, :])
            nc.sync.dma_start(out=lng[:, :], in_=lng_in[None, :])
            nc.sync.dma_start(out=lnb[:, :], in_=lnb_in[None, :])
            nc.sync.dma_start(out=g_row[:, C:], in_=gdin[None, :])
            if with_bias:
                nc.sync.dma_start(out=cbx[:, :], in_=cbx_in[:, :])


            # ---- global-pool partials per slab ----
            for s in range(NSLAB):
                v.reduce_sum(
                    part[:, s : s + 1],
                    xs[:, s * HW : (s + 1) * HW],
                    axis=mybir.AxisListType.X,
                )
            # cross-partition (c,d)->channel sum via select matmuls
            fps = ppool.tile([48, 1], F32, tag="featps")
            for s in range(NSLAB):
                nc.tensor.matmul(
                    fps[:, :],
                    csel[:, s * 48 : (s + 1) * 48],
                    part[:, s : s + 1],
                    start=(s == 0),
                    stop=(s == NSLAB - 1),
                    skip_group_check=True,
                )
            sc.activation(featp[:, :], fps[:, :], ACTF.Copy, scale=1.0 / (D * HW))

            # ---- AllGather (pairwise) of the [96] pooled features ----
            # each core contributes its own 48 channels; within a pair the
            # rank order [2b, 2b+1] = [hc0, hc1] concatenates them in
            # channel order on both cores.
            nc.sync.dma_start(out=cin[:, :], in_=featp[:, :])
            nc.gpsimd.collective_compute(
                "AllGather",
                ALU.bypass,
                replica_groups=[[2 * b, 2 * b + 1] for b in range(B)],
                ins=[cin.opt()],
                outs=[cout.opt()],
            )
            nc.sync.dma_start(
                out=g_row[:, 0:C],
                in_=cout[:, :].rearrange("c one -> one c"),
            )

            # ---- LayerNorm over 192 on one partition ----
            v.reduce_sum(s1[:, :], g_row[:, :], axis=mybir.AxisListType.X)
            v.tensor_scalar_mul(s1[:, :], s1[:, :], 1.0 / (C + G))  # mu
            v.tensor_scalar(
                out=gd_row[:, :], in0=g_row[:, :], scalar1=s1[:, :], scalar2=None,
                op0=ALU.subtract,
            )
            v.tensor_tensor(out=gn_row[:, :], in0=gd_row[:, :], in1=gd_row[:, :], op=ALU.mult)
            v.reduce_sum(s2[:, :], gn_row[:, :], axis=mybir.AxisListType.X)
            v.tensor_scalar(
                out=s2[:, :], in0=s2[:, :], scalar1=1.0 / (C + G), scalar2=LN_EPS,
                op0=ALU.mult, op1=ALU.add,
            )  # var + eps
            sc.activation(s3[:, :], s2[:, :], ACTF.Sqrt)
            # one Newton step for a clean sqrt
            v.reciprocal(s4[:, :], s3[:, :])
            v.tensor_tensor(out=s4[:, :], in0=s4[:, :], in1=s2[:, :], op=ALU.mult)
            v.tensor_tensor(out=s4[:, :], in0=s4[:, :], in1=s3[:, :], op=ALU.add)
            v.tensor_scalar_mul(s4[:, :], s4[:, :], 0.5)
            v.reciprocal(s3[:, :], s4[:, :])  # rstd
            v.tensor_scalar(
                out=gn_row[:, :], in0=gd_row[:, :], scalar1=s3[:, :], scalar2=None,
                op0=ALU.mult,
            )
            v.tensor_tensor(out=gn_row[:, :], in0=gn_row[:, :], in1=lng[:, :], op=ALU.mult)
            v.tensor_tensor(out=gn_row[:, :], in0=gn_row[:, :], in1=lnb[:, :], op=ALU.add)

            # ---- MLP layer 1: h = gelu(gn @ w1 + b1) via row-products ----
            nc.sync.dma_start(out=gb[:, :], in_=gn_row[:, :])
            nc.sync.dma_start(out=gn_bc[:, :], in_=gb[:1, :].partition_broadcast(HID))
            v.tensor_tensor(out=prod[:, :], in0=w1t[:, :], in1=gn_bc[:, :], op=ALU.mult)
            v.reduce_sum(hvec[:, :], prod[:, :], axis=mybir.AxisListType.X)
            v.tensor_tensor(out=hvec[:, :], in0=hvec[:, :], in1=b1c[:, :], op=ALU.add)
            sc.activation(hvec[:, :], hvec[:, :], ACTF.Gelu)

            # ---- MLP layer 2 via DRAM transpose bounce ----
            v.tensor_scalar(
                out=l2tmp[:, :], in0=w2t[:, :], scalar1=hvec[:, :], scalar2=None,
                op0=ALU.mult,
            )
            nc.sync.dma_start(out=zt[:, :], in_=l2tmp[:, :])
            nc.sync.dma_start(out=z72[:, :], in_=zt[:, :])
            z3 = z72[:, :].rearrange("a (j i) -> a j i", j=HID, i=NB)
            for i in range(NB):
                v.reduce_sum(zrow[:, i : i + 1], z3[:, :, i], axis=mybir.AxisListType.X)
            v.tensor_tensor(out=zrow[:, :], in0=zrow[:, :], in1=b2r[:, :], op=ALU.add)

            # ---- softmax over 3 ----
            v.reduce_max(s1[:, :], zrow[:, :], axis=mybir.AxisListType.X)
            v.tensor_scalar(
                out=zrow[:, :], in0=zrow[:, :], scalar1=s1[:, :], scalar2=None,
                op0=ALU.subtract,
            )
            sc.activation(zrow[:, :], zrow[:, :], ACTF.Exp)
            v.reduce_sum(s2[:, :], zrow[:, :], axis=mybir.AxisListType.X)
            v.reciprocal(s2[:, :], s2[:, :])
            v.tensor_scalar(
                out=wts[:, :], in0=zrow[:, :], scalar1=s2[:, :], scalar2=None,
                op0=ALU.mult,
            )

            # ---- fold gate weights into per-channel tap weights ----
            nc.sync.dma_start(out=wb[:, :], in_=wts[:, :])
            nc.sync.dma_start(out=wts_bc[:, :], in_=wb[:1, :].partition_broadcast(NP))
            for s in range(NSLAB):
                for i in range(NB):
                    sl = slice(s * 81 + i * 27, s * 81 + (i + 1) * 27)
                    v.tensor_scalar(
                        out=w_exp[:, sl], in0=cwx[:, sl],
                        scalar1=wts_bc[:, i : i + 1], scalar2=None, op0=ALU.mult,
                    )
            if with_bias:
                for i in range(NB):
                    v.tensor_scalar(
                        out=betmp[:, i::NB], in0=cbx[:, i::NB],
                        scalar1=wts_bc[:, i : i + 1], scalar2=None, op0=ALU.mult,
                    )
                v.tensor_tensor(
                    out=b_exp[:, :], in0=betmp[:, 0::NB], in1=betmp[:, 1::NB],
                    op=ALU.add,
                )
                v.tensor_tensor(
                    out=b_exp[:, :], in0=b_exp[:, :], in1=betmp[:, 2::NB],
                    op=ALU.add,
                )

            # ---- the conv: per slab, 25 band-matmul passes over 7 chunks ----
            for s in range(NSLAB):
                # build the 25 band matrices for this slab (DVE)
                mats = []
                for mi, (oh, ow, entries) in enumerate(passes):
                    mt = matpool.tile([NP, NP], BF16, tag=f"m{mi}")
                    for ei, (od, t) in enumerate(entries):
                        mk_in = masks[:, (od + 3) * NP : (od + 4) * NP]
                        wcol = w_exp[:, s * 81 + t : s * 81 + t + 1]
                        if ei == 0:
                            v.tensor_scalar(
                                out=mt[:, :], in0=mk_in, scalar1=wcol,
                                scalar2=None, op0=ALU.mult,
                            )
                        else:
                            v.scalar_tensor_tensor(
                                out=mt[:, :], in0=mk_in, scalar=wcol,
                                in1=mt[:, :], op0=ALU.mult, op1=ALU.add,
                            )
                    mats.append(mt)

                xf = xs[:, s * HW : (s + 1) * HW]
                xv = xf.rearrange("p (h w) -> p h w", h=H, w=W)
                for ci in range(N_CHUNKS):
                    ps = ppool.tile([NP, CHUNK], F32, tag=f"ps{ci}")
                    pv = ps[:, :].rearrange("p (h w) -> p h w", h=ROWS, w=W)
                    nmm = 0
                    for mi, (oh, ow, entries) in enumerate(passes):
                        if mi == 0:
                            nc.tensor.matmul(
                                ps[:, :], mats[0][:, :],
                                xf[:, ci * CHUNK : (ci + 1) * CHUNK],
                                start=True, stop=False, skip_group_check=True,
                            )
                            nmm += 1
                            continue
                        h0 = max(ci * ROWS, -oh if oh < 0 else 0)
                        h1 = min(ci * ROWS + ROWS, H - (oh if oh > 0 else 0))
                        if h1 <= h0:
                            continue
                        w0 = -ow if ow < 0 else 0
                        w1 = W - (ow if ow > 0 else 0)
                        nc.tensor.matmul(
                            pv[:, h0 - ci * ROWS : h1 - ci * ROWS, w0:w1],
                            mats[mi][:, :],
                            xv[:, h0 + oh : h1 + oh, w0 + ow : w1 + ow],
                            start=False, stop=(mi == len(passes) - 1),
                            skip_group_check=True,
                        )
                        nmm += 1
                    ot = outpool.tile([NP, CHUNK], F32, tag=f"o{ci % 4}")
                    sc.activation(ot[:, :], ps[:, :], ACTF.Copy)
                    if with_bias:
                        v.tensor_scalar(
                            out=ot[:, :], in0=ot[:, :],
                            scalar1=b_exp[:, s : s + 1], scalar2=None,
                            op0=ALU.add,
                        )
                    nc.sync.dma_start(
                        out=yout[:, s * HW + ci * CHUNK : s * HW + (ci + 1) * CHUNK],
                        in_=ot[:, :],
                    )

    _split_sem_waits(nc)
    return nc


_WAITSPLIT = [0]


def _split_sem_waits(nc, max_waits=1):
    """This walrus build rejects >1 SyncWait per instruction (and any wait on
    a Drain). Move excess waits onto same-engine NOPs inserted just before."""
    for bb in nc.main_func.blocks:
        insns = bb.instructions
        i = 0
        while i < len(insns):
            ins = insns[i]
            si = ins.sync_info
            limit = 0 if ins.opcode == "Drain" else max_waits
            if si is not None and si.on_wait is not None and len(si.on_wait) > limit:
                waits = list(si.on_wait)
                keep = waits[-limit:] if limit else []
                extra = waits[: len(waits) - limit]
                pos = i
                for j in range(0, len(extra), max_waits):
                    nop = mybir.InstNoOp(
                        name=f"I-waitsplit-{_WAITSPLIT[0]}", ins=[], outs=[]
                    )
                    _WAITSPLIT[0] += 1
                    nop.engine = ins.engine
                    nop.sync_info = mybir.SyncInfo(
                        on_wait=extra[j : j + max_waits], on_update=[]
                    )
                    insns.insert(pos, nop)
                    pos += 1
                    i += 1
                si.on_wait = keep
            i += 1


def _make_masks():
    m = np.zeros((NP, 7 * NP), dtype=np.float32)
    for od in range(-3, 4):
        for p in range(NP):
            q = p - od
            if q // D == p // D and 0 <= q < NP:
                m[p, (od + 3) * NP + q] = 1.0
    return m.astype(ml_dtypes.bfloat16)


def _prep_inputs(x, guidance, convw, convb, ln_g, ln_b, w1, b1, w2, b2):
    f = np.float32
    w3 = np.ascontiguousarray(convw.reshape(NB, C, 27), dtype=f)
    cb = np.ascontiguousarray(convb, dtype=f)
    masks = _make_masks()
    csel = np.zeros((NP, NSLAB * 48), dtype=f)
    for p in range(NP):
        c = p // D
        for s in range(NSLAB):
            csel[p, s * 48 + s * NCH + c] = 1.0
    common = dict(
        masks=masks,
        csel=csel,
        zr48=np.zeros((48, 1), dtype=f),
        w1t=np.ascontiguousarray(w1.T, dtype=f),
        b1=np.ascontiguousarray(b1, dtype=f),
        w2=np.ascontiguousarray(w2, dtype=f),
        b2=np.ascontiguousarray(b2, dtype=f),
        lng=np.ascontiguousarray(ln_g, dtype=f),
        lnb=np.ascontiguousarray(ln_b, dtype=f),
    )
    in_maps = []
    for core in range(N_CORES):
        b, hc = core // 2, core % 2
        ch0 = 48 * hc
        # xs[p=c*16+d, s*HW+j] = x[b, ch0+8s+c, d, j]
        arr = np.ascontiguousarray(x[b, ch0 : ch0 + 48], dtype=f)
        arr = arr.reshape(NSLAB, NCH, D, HW).transpose(1, 2, 0, 3).reshape(
            NP, NSLAB * HW
        )
        # cwx[p=c*16+d, s*81+t] = convw[br, ch0+8s+c, t27]  (d-independent)
        cw = w3[:, ch0 : ch0 + 48, :].reshape(NB, NSLAB, NCH, 27)
        cw = cw.transpose(2, 1, 0, 3).reshape(NCH, NSLAB * NB * 27)  # [c, s*81]
        cwx = np.repeat(cw, D, axis=0)  # row c*16+d <- cw[c]
        cbx = cb[:, ch0 : ch0 + 48].T.reshape(NSLAB, NCH, NB).transpose(1, 0, 2)
        cbx = np.repeat(cbx, D, axis=0).reshape(NCH, D, NSLAB, NB).reshape(
            NP, NSLAB * NB
        )
        hcs = np.zeros((1, 2), dtype=f)
        hcs[0, hc] = 1.0
        in_maps.append(
            dict(
                x=arr.astype(ml_dtypes.bfloat16),
                gd=np.ascontiguousarray(guidance[b], dtype=f),
                cwx=np.ascontiguousarray(cwx, dtype=f),
                cbx=np.ascontiguousarray(cbx, dtype=f),
                hcsel=hcs,
                **common,
            )
        )
    return in_maps


_CACHED_NC = {}


def kernel(x, guidance, convw, convb, ln_g, ln_b, w1, b1, w2, b2):
    with_bias = bool(np.any(np.asarray(convb)))
    if with_bias not in _CACHED_NC:
        _CACHED_NC[with_bias] = _build_program(with_bias)
    nc = _CACHED_NC[with_bias]
    globals()["_LAST_NC"] = nc
    in_maps = _prep_inputs(
        x, guidance, convw, convb, ln_g, ln_b, w1, b1, w2, b2
    )
    res = run_bass_kernel_spmd(nc, in_maps, list(range(N_CORES)))
    out = np.empty((B, C, D, H, W), dtype=np.float32)
    for core in range(N_CORES):
        b, hc = core // 2, core % 2
        y = res.results[core]["y"].reshape(NCH, D, NSLAB, HW)
        out[b, 48 * hc : 48 * hc + 48] = (
            y.transpose(2, 0, 1, 3).reshape(48, D, H, W)
        )
    return out


if __name__ == "__main__":
    rng = np.random.default_rng(0)
    ins = dict(
        x=rng.standard_normal((B, C, D, H, W), dtype=np.float32),
        guidance=rng.standard_normal((B, G), dtype=np.float32),
        convw=(rng.standard_normal((NB, C, 1, K, K, K)) * 0.1).astype(np.float32),
        convb=np.zeros((NB, C), np.float32),
        ln_g=np.ones((C + G,), np.float32),
        ln_b=np.zeros((C + G,), np.float32),
        w1=(rng.standard_normal((C + G, HID)) * 0.05).astype(np.float32),
        b1=np.zeros((HID,), np.float32),
        w2=(rng.standard_normal((HID, NB)) * 0.05).astype(np.float32),
        b2=np.zeros((NB,), np.float32),
    )
    out = kernel(**ins)
    print("kernel ran, out shape", out.shape, "mean", float(np.abs(out).mean()))


# revision 11
# speedup vs baseline: 3.3556x; 1.1542x over previous
"""AttentionGuidedDynamicRangeDWConv3D on 8 Trainium2 NeuronCores.

Module: out = sum_i softmax(MLP(LN([mean_dhw(x), guidance])))[:, i]
                * dwconv3d(x, convw[i], convb[i], dil=i+1)
Shapes: x [4,96,16,56,56] f32, 3 branches of 3x3x3 depthwise conv with
dilations 1/2/3 ('same' zero padding).

Sharding: 8 cores = (batch b in 0..3) x (channel half hc in 0..1); each
core owns 48 channels of one batch at FULL depth.

Layout trick: partitions = (channel c in 0..8) x (depth d in 0..16), so a
single bf16 matmul with a 128x128 block-banded weight matrix applies an
entire depth-band of conv taps at once: out[(c,d), hw] +=
sum_od w[c, (od,oh,ow)] * x[(c,d+od), hw + oh*56+ow].  The 81 taps
(3 branches x 27) collapse into 25 matmul passes -- one per distinct
(oh,ow) pair -- accumulated in PSUM per 448-column (8 h-row) chunk.
Depth 'same' padding falls out of band truncation (no halo).  H/W 'same'
padding is exact via trimmed 2D access patterns (bf16 matmuls allow
strided APs; fp32r would not).

Band matrices are built by the Vector engine from host-supplied
shifted-identity masks scaled by per-partition weight columns.  The gate
MLP runs redundantly per core; the global mean-pool takes one pairwise
96x6-float AllGather (15us fixed latency in the cost model).  To hide
that latency plus the MLP, slabs 0 and 1 run UNGATED: their 27
single-branch passes accumulate the three branch convs into separate
PSUM banks, Act copies them to SBUF, and once the softmax weights are
ready the Vector engine does the weighted 3-way merge.  Slabs 2-5 use
gate-folded matrices (25 passes) and a plain Act PSUM->SBUF copy.
Engine streams are in-order, so emission order is chosen to keep DVE
(matrix builds) and Act (pool reductions + PSUM copies) ahead of the
Tensor engine throughout.
"""

import sys

if "/opt/trn_rl_repo" not in sys.path:
    sys.path.insert(0, "/opt/trn_rl_repo")

import ml_dtypes
import numpy as np

import concourse.bass as bass
import concourse.mybir as mybir
import concourse.tile as tile
from concourse.bass_utils import run_bass_kernel_spmd

F32 = mybir.dt.float32
BF16 = mybir.dt.bfloat16
ALU = mybir.AluOpType
ACTF = mybir.ActivationFunctionType

B, C, D, H, W = 4, 96, 16, 56, 56
G, HID, NB = 96, 24, 3
K = 3
DILS = (1, 2, 3)
LN_EPS = 1e-5
N_CORES = 8
NCH = 8                  # channels per slab
NSLAB = 6                # slabs per core (48 channels)
NUNG = 2                 # ungated slabs (hide the collective+MLP latency)
NP = NCH * D             # 128 partitions
HW = H * W               # 3136
CHUNK = 448              # 8 h-rows; PSUM bank-sized chunk
N_CHUNKS = 7
ROWS = CHUNK // W        # 8


def _pass_list(split_branches):
    """[(oh, ow, [(od, t), ...])].  split_branches: one pass per (branch,
    (oh,ow)) with the branch's (0,0) pass first (27 passes); else one pass
    per distinct (oh,ow) with (0,0) merged across branches first (25)."""
    out = []
    for i, d in enumerate(DILS):
        for oh in (0, -d, d):
            for ow in (0, -d, d) if oh == 0 else (-d, 0, d):
                if split_branches:
                    kh, kw = oh // d + 1, ow // d + 1
                    ents = [(kd * d - d, i * 27 + kd * 9 + kh * 3 + kw)
                            for kd in range(K)]
                    out.append((oh, ow, ents))
                else:
                    if (oh, ow) == (0, 0) and i > 0:
                        continue
                    ents = []
                    for j, dj in enumerate(DILS):
                        if oh in (-dj, 0, dj) and ow in (-dj, 0, dj):
                            kh, kw = oh // dj + 1, ow // dj + 1
                            ents += [(kd * dj - dj,
                                      j * 27 + kd * 9 + kh * 3 + kw)
                                     for kd in range(K)]
                    out.append((oh, ow, ents))
    if split_branches:
        assert len(out) == 27
    else:
        assert len(out) == 25
    assert sum(len(e) for _, _, e in out) == 81
    return out


def _build_program(with_bias):
    nc = bass.Bass()
    xin = nc.dram_tensor("x", [NP, NSLAB * HW], BF16, kind="ExternalInput")
    masks_in = nc.dram_tensor("masks", [NP, 7 * NP], BF16, kind="ExternalInput")
    cwx_in = nc.dram_tensor("cwx", [NP, NSLAB * 27 * NB], F32, kind="ExternalInput")
    cbx_in = nc.dram_tensor("cbx", [NP, NSLAB * NB], F32, kind="ExternalInput")
    gdin = nc.dram_tensor("gd", [G], F32, kind="ExternalInput")
    w1t_in = nc.dram_tensor("w1t", [HID, C + G], F32, kind="ExternalInput")
    b1_in = nc.dram_tensor("b1", [HID], F32, kind="ExternalInput")
    w2_in = nc.dram_tensor("w2", [HID, NB], F32, kind="ExternalInput")
    b2_in = nc.dram_tensor("b2", [NB], F32, kind="ExternalInput")
    lng_in = nc.dram_tensor("lng", [C + G], F32, kind="ExternalInput")
    lnb_in = nc.dram_tensor("lnb", [C + G], F32, kind="ExternalInput")
    yout = nc.dram_tensor("y", [NP, NSLAB * HW], F32, kind="ExternalOutput")

    p_ung = _pass_list(True)
    p_gat = _pass_list(False)

    with tile.TileContext(nc) as tc:
        with (
            tc.tile_pool(name="sbuf", bufs=1) as pool,
            tc.tile_pool(name="mats", bufs=2) as matpool,
            tc.tile_pool(name="outs", bufs=4) as outpool,
            tc.tile_pool(name="dram", bufs=1, space="DRAM") as dpool,
            tc.tile_pool(name="psum", bufs=1, space="PSUM") as ppool,
        ):
            xs = pool.tile([NP, NSLAB * HW], BF16, tag="xs")
            masks = pool.tile([NP, 7 * NP], BF16, tag="masks")
            cwx = pool.tile([NP, NSLAB * 27 * NB], F32, tag="cwx")
            w_exp = pool.tile([NP, NSLAB * 27 * NB], F32, tag="w_exp")
            scr = pool.tile([NP, HW], BF16, tag="scr")
            part = pool.tile([NP, NSLAB], F32, tag="part")
            grow = pool.tile([1, 2 * NP * NSLAB], F32, tag="grow")
            bb = [
                [
                    pool.tile([NP, HW], F32, tag=f"bb{s}_{b}",
                              name=f"bb{s}_{b}")
                    for b in range(NB)
                ]
                for s in range(NUNG)
            ]
            g_row = pool.tile([1, C + G], F32, tag="g_row")
            gd_row = pool.tile([1, C + G], F32, tag="gd_row")
            lng = pool.tile([1, C + G], F32, tag="lng")
            lnb = pool.tile([1, C + G], F32, tag="lnb")
            gn_row = pool.tile([1, C + G], F32, tag="gn_row")
            gn_bc = pool.tile([HID, C + G], F32, tag="gn_bc")
            w1t = pool.tile([HID, C + G], F32, tag="w1t")
            prod = pool.tile([HID, C + G], F32, tag="prod")
            hvec = pool.tile([HID, 1], F32, tag="hvec")
            b1c = pool.tile([HID, 1], F32, tag="b1c")
            w2t = pool.tile([HID, NB], F32, tag="w2t")
            l2tmp = pool.tile([HID, NB], F32, tag="l2tmp")
            z72 = pool.tile([1, HID * NB], F32, tag="z72")
            zrow = pool.tile([1, NB], F32, tag="zrow")
            b2r = pool.tile([1, NB], F32, tag="b2r")
            wts = pool.tile([1, NB], F32, tag="wts")
            wts_bc = pool.tile([NP, NB], F32, tag="wts_bc")
            s1 = pool.tile([1, 1], F32, tag="s1")
            s2 = pool.tile([1, 1], F32, tag="s2")
            s3 = pool.tile([1, 1], F32, tag="s3")
            s4 = pool.tile([1, 1], F32, tag="s4")
            if with_bias:
                cbx = pool.tile([NP, NSLAB * NB], F32, tag="cbx")
                b_exp = pool.tile([NP, NSLAB], F32, tag="b_exp")
                betmp = pool.tile([NP, NSLAB * NB], F32, tag="betmp")

            cin = dpool.tile([NP, NSLAB], F32, tag="cin")
            cout = dpool.tile([2 * NP, NSLAB], F32, tag="cout")
            zt = dpool.tile([1, HID * NB], F32, tag="zt")
            gb = dpool.tile([1, C + G], F32, tag="gb")
            wb = dpool.tile([1, NB], F32, tag="wb")

            v = nc.vector
            sc = nc.scalar

            # ---- A: loads (small weights first, then x slab-by-slab) ----
            nc.sync.dma_start(out=masks[:, :], in_=masks_in[:, :])
            nc.sync.dma_start(out=cwx[:, :], in_=cwx_in[:, :])
            nc.sync.dma_start(out=w1t[:, :], in_=w1t_in[:, :])
            nc.sync.dma_start(out=b1c[:, :], in_=b1_in[:, None])
            nc.sync.dma_start(out=w2t[:, :], in_=w2_in[:, :])
            nc.sync.dma_start(out=b2r[:, :], in_=b2_in[None, :])
            nc.sync.dma_start(out=lng[:, :], in_=lng_in[None, :])
            nc.sync.dma_start(out=lnb[:, :], in_=lnb_in[None, :])
            nc.sync.dma_start(out=g_row[:, C:], in_=gdin[None, :])
            if with_bias:
                nc.sync.dma_start(out=cbx[:, :], in_=cbx_in[:, :])
            for s in range(NSLAB):
                nc.sync.dma_start(
                    out=xs[:, s * HW : (s + 1) * HW],
                    in_=xin[:, s * HW : (s + 1) * HW],
                )

            # ---- B: per-slab plane sums on the Act engine ----
            for s in range(NSLAB):
                sc.activation(
                    scr[:, :], xs[:, s * HW : (s + 1) * HW], ACTF.Copy,
                    accum_out=part[:, s : s + 1],
                )

            # ---- C: pairwise AllGather of raw plane sums ----
            nc.sync.dma_start(out=cin[:, :], in_=part[:, :])
            nc.gpsimd.collective_compute(
                "AllGather",
                ALU.bypass,
                replica_groups=[[2 * b, 2 * b + 1] for b in range(B)],
                ins=[cin.opt()],
                outs=[cout.opt()],
            )
            nc.sync.dma_start(out=grow[:, :], in_=cout[:, :])

            # ---- conv helpers ----
            def build_mats(s, passes, wsrc):
                mats = []
                for mi, (oh, ow, entries) in enumerate(passes):
                    mt = matpool.tile([NP, NP], BF16, tag=f"m{mi}")
                    for ei, (od, t) in enumerate(entries):
                        mk_in = masks[:, (od + 3) * NP : (od + 4) * NP]
                        wcol = wsrc[:, s * 81 + t : s * 81 + t + 1]
                        if ei == 0:
                            v.tensor_scalar(
                                out=mt[:, :], in0=mk_in, scalar1=wcol,
                                scalar2=None, op0=ALU.mult,
                            )
                        else:
                            v.scalar_tensor_tensor(
                                out=mt[:, :], in0=mk_in, scalar=wcol,
                                in1=mt[:, :], op0=ALU.mult, op1=ALU.add,
                            )
                    mats.append(mt)
                return mats

            def emit_pass(ps, pv, mt, xf, xv, ci, oh, ow, start, stop):
                if (oh, ow) == (0, 0):
                    nc.tensor.matmul(
                        ps[:, :], mt[:, :],
                        xf[:, ci * CHUNK : (ci + 1) * CHUNK],
                        start=start, stop=stop, skip_group_check=True,
                    )
                    return
                h0 = max(ci * ROWS, -oh if oh < 0 else 0)
                h1 = min(ci * ROWS + ROWS, H - (oh if oh > 0 else 0))
                if h1 <= h0:
                    return
                w0 = -ow if ow < 0 else 0
                w1 = W - (ow if ow > 0 else 0)
                nc.tensor.matmul(
                    pv[:, h0 - ci * ROWS : h1 - ci * ROWS, w0:w1],
                    mt[:, :],
                    xv[:, h0 + oh : h1 + oh, w0 + ow : w1 + ow],
                    start=start, stop=stop, skip_group_check=True,
                )

            def slab_views(s):
                xf = xs[:, s * HW : (s + 1) * HW]
                return xf, xf.rearrange("p (h w) -> p h w", h=H, w=W)

            def emit_ungated_matmuls(s, mats):
                xf, xv = slab_views(s)
                pss = {}
                for ci in range(N_CHUNKS):
                    for b in range(NB):
                        ps = ppool.tile([NP, CHUNK], F32,
                                        tag=f"ps{(3 * ci + b) % 7}")
                        pss[(ci, b)] = ps
                        pv = ps[:, :].rearrange("p (h w) -> p h w", h=ROWS, w=W)
                        for k in range(9):
                            oh, ow, _ = p_ung[b * 9 + k]
                            emit_pass(ps, pv, mats[b * 9 + k], xf, xv, ci,
                                      oh, ow, k == 0, k == 8)
                return pss

            def emit_ungated_copies(s, pss):
                for ci in range(N_CHUNKS):
                    for b in range(NB):
                        sc.activation(
                            bb[s][b][:, ci * CHUNK : (ci + 1) * CHUNK],
                            pss[(ci, b)][:, :], ACTF.Copy,
                        )

            def emit_merge(s):
                for ci in range(N_CHUNKS):
                    sl = slice(ci * CHUNK, (ci + 1) * CHUNK)
                    ot = outpool.tile([NP, CHUNK], F32, tag=f"o{ci % 4}")
                    v.tensor_scalar(
                        out=ot[:, :], in0=bb[s][0][:, sl],
                        scalar1=wts_bc[:, 0:1], scalar2=None, op0=ALU.mult,
                    )
                    for b in (1, 2):
                        v.scalar_tensor_tensor(
                            out=ot[:, :], in0=bb[s][b][:, sl],
                            scalar=wts_bc[:, b : b + 1], in1=ot[:, :],
                            op0=ALU.mult, op1=ALU.add,
                        )
                    if with_bias:
                        v.tensor_scalar(
                            out=ot[:, :], in0=ot[:, :],
                            scalar1=b_exp[:, s : s + 1], scalar2=None,
                            op0=ALU.add,
                        )
                    nc.sync.dma_start(
                        out=yout[:, s * HW + ci * CHUNK : s * HW + (ci + 1) * CHUNK],
                        in_=ot[:, :],
                    )

            def emit_gated_slab(s):
                mats = build_mats(s, p_gat, w_exp)
                xf, xv = slab_views(s)
                for ci in range(N_CHUNKS):
                    ps = ppool.tile([NP, CHUNK], F32, tag=f"ps{ci}")
                    pv = ps[:, :].rearrange("p (h w) -> p h w", h=ROWS, w=W)
                    for mi, (oh, ow, _) in enumerate(p_gat):
                        emit_pass(ps, pv, mats[mi], xf, xv, ci, oh, ow,
                                  mi == 0, mi == len(p_gat) - 1)
                    ot = outpool.tile([NP, CHUNK], F32, tag=f"o{ci % 4}")
                    sc.activation(ot[:, :], ps[:, :], ACTF.Copy)
                    if with_bias:
                        v.tensor_scalar(
                            out=ot[:, :], in0=ot[:, :],
                            scalar1=b_exp[:, s : s + 1], scalar2=None,
                            op0=ALU.add,
                        )
                    nc.sync.dma_start(
                        out=yout[:, s * HW + ci * CHUNK : s * HW + (ci + 1) * CHUNK],
                        in_=ot[:, :],
                    )

            # ---- D: slab 0 ungated (builds + matmuls + copies) ----
            mats0 = build_mats(0, p_ung, cwx)
            pss0 = emit_ungated_matmuls(0, mats0)
            emit_ungated_copies(0, pss0)

            # ---- F1: slab 1 ungated (builds + matmuls; copies later) ----
            mats1 = build_mats(1, p_ung, cwx)
            pss1 = emit_ungated_matmuls(1, mats1)

            # ---- E: gate MLP ----
            # feat[48r + 8s + c] = sum_d cout[r, (c,d), s] / (D*HW)
            for r in range(2):
                gview = grow[:, r * NP * NSLAB : (r + 1) * NP * NSLAB].rearrange(
                    "a (c d s) -> a s c d", c=NCH, d=D, s=NSLAB
                )
                tview = g_row[:, 48 * r : 48 * r + 48].rearrange(
                    "a (s c) -> a s c", s=NSLAB, c=NCH
                )
                v.reduce_sum(tview, gview, axis=mybir.AxisListType.X)
            v.tensor_scalar_mul(g_row[:, 0:C], g_row[:, 0:C], 1.0 / (D * HW))

            # LayerNorm over 192 on one partition
            v.reduce_sum(s1[:, :], g_row[:, :], axis=mybir.AxisListType.X)
            v.tensor_scalar_mul(s1[:, :], s1[:, :], 1.0 / (C + G))  # mu
            v.tensor_scalar(
                out=gd_row[:, :], in0=g_row[:, :], scalar1=s1[:, :], scalar2=None,
                op0=ALU.subtract,
            )
            v.tensor_tensor(out=gn_row[:, :], in0=gd_row[:, :], in1=gd_row[:, :], op=ALU.mult)
            v.reduce_sum(s2[:, :], gn_row[:, :], axis=mybir.AxisListType.X)
            v.tensor_scalar(
                out=s2[:, :], in0=s2[:, :], scalar1=1.0 / (C + G), scalar2=LN_EPS,
                op0=ALU.mult, op1=ALU.add,
            )  # var + eps
            sc.activation(s3[:, :], s2[:, :], ACTF.Sqrt)
            # one Newton step for a clean sqrt
            v.reciprocal(s4[:, :], s3[:, :])
            v.tensor_tensor(out=s4[:, :], in0=s4[:, :], in1=s2[:, :], op=ALU.mult)
            v.tensor_tensor(out=s4[:, :], in0=s4[:, :], in1=s3[:, :], op=ALU.add)
            v.tensor_scalar_mul(s4[:, :], s4[:, :], 0.5)
            v.reciprocal(s3[:, :], s4[:, :])  # rstd
            v.tensor_scalar(
                out=gn_row[:, :], in0=gd_row[:, :], scalar1=s3[:, :], scalar2=None,
                op0=ALU.mult,
            )
            v.tensor_tensor(out=gn_row[:, :], in0=gn_row[:, :], in1=lng[:, :], op=ALU.mult)
            v.tensor_tensor(out=gn_row[:, :], in0=gn_row[:, :], in1=lnb[:, :], op=ALU.add)

            # MLP layer 1: h = gelu(gn @ w1 + b1) via row-products
            nc.sync.dma_start(out=gb[:, :], in_=gn_row[:, :])
            nc.sync.dma_start(out=gn_bc[:, :], in_=gb[:1, :].partition_broadcast(HID))
            v.tensor_tensor(out=prod[:, :], in0=w1t[:, :], in1=gn_bc[:, :], op=ALU.mult)
            v.reduce_sum(hvec[:, :], prod[:, :], axis=mybir.AxisListType.X)
            v.tensor_tensor(out=hvec[:, :], in0=hvec[:, :], in1=b1c[:, :], op=ALU.add)
            sc.activation(hvec[:, :], hvec[:, :], ACTF.Gelu)

            # MLP layer 2 via DRAM transpose bounce
            v.tensor_scalar(
                out=l2tmp[:, :], in0=w2t[:, :], scalar1=hvec[:, :], scalar2=None,
                op0=ALU.mult,
            )
            nc.sync.dma_start(out=zt[:, :], in_=l2tmp[:, :])
            nc.sync.dma_start(out=z72[:, :], in_=zt[:, :])
            z3 = z72[:, :].rearrange("a (j i) -> a j i", j=HID, i=NB)
            for i in range(NB):
                v.reduce_sum(zrow[:, i : i + 1], z3[:, :, i], axis=mybir.AxisListType.X)
            v.tensor_tensor(out=zrow[:, :], in0=zrow[:, :], in1=b2r[:, :], op=ALU.add)

            # softmax over 3
            v.reduce_max(s1[:, :], zrow[:, :], axis=mybir.AxisListType.X)
            v.tensor_scalar(
                out=zrow[:, :], in0=zrow[:, :], scalar1=s1[:, :], scalar2=None,
                op0=ALU.subtract,
            )
            sc.activation(zrow[:, :], zrow[:, :], ACTF.Exp)
            v.reduce_sum(s2[:, :], zrow[:, :], axis=mybir.AxisListType.X)
            v.reciprocal(s2[:, :], s2[:, :])
            v.tensor_scalar(
                out=wts[:, :], in0=zrow[:, :], scalar1=s2[:, :], scalar2=None,
                op0=ALU.mult,
            )

            # broadcast gate weights; fold into per-channel tap weights
            nc.sync.dma_start(out=wb[:, :], in_=wts[:, :])
            nc.sync.dma_start(out=wts_bc[:, :], in_=wb[:1, :].partition_broadcast(NP))
            for s in range(NUNG, NSLAB):
                for i in range(NB):
                    sl = slice(s * 81 + i * 27, s * 81 + (i + 1) * 27)
                    v.tensor_scalar(
                        out=w_exp[:, sl], in0=cwx[:, sl],
                        scalar1=wts_bc[:, i : i + 1], scalar2=None, op0=ALU.mult,
                    )
            if with_bias:
                for i in range(NB):
                    v.tensor_scalar(
                        out=betmp[:, i::NB], in0=cbx[:, i::NB],
                        scalar1=wts_bc[:, i : i + 1], scalar2=None, op0=ALU.mult,
                    )
                v.tensor_tensor(
                    out=b_exp[:, :], in0=betmp[:, 0::NB], in1=betmp[:, 1::NB],
                    op=ALU.add,
                )
                v.tensor_tensor(
                    out=b_exp[:, :], in0=b_exp[:, :], in1=betmp[:, 2::NB],
                    op=ALU.add,
                )

            # ---- F2: slab 1 copies (Act; after the MLP's Act ops) ----
            emit_ungated_copies(1, pss1)

            # ---- G: slab 0 weighted merge + store ----
            emit_merge(0)

            # ---- H..J: gated slabs, with slab-1 merge interleaved ----
            emit_gated_slab(2)
            emit_merge(1)
            for s in range(3, NSLAB):
                emit_gated_slab(s)

    _split_sem_waits(nc)
    return nc


_WAITSPLIT = [0]


def _split_sem_waits(nc, max_waits=1):
    """This walrus build rejects >1 SyncWait per instruction (and any wait on
    a Drain). Move excess waits onto same-engine NOPs inserted just before."""
    for bb in nc.main_func.blocks:
        insns = bb.instructions
        i = 0
        while i < len(insns):
            ins = insns[i]
            si = ins.sync_info
            limit = 0 if ins.opcode == "Drain" else max_waits
            if si is not None and si.on_wait is not None and len(si.on_wait) > limit:
                waits = list(si.on_wait)
                keep = waits[-limit:] if limit else []
                extra = waits[: len(waits) - limit]
                pos = i
                for j in range(0, len(extra), max_waits):
                    nop = mybir.InstNoOp(
                        name=f"I-waitsplit-{_WAITSPLIT[0]}", ins=[], outs=[]
                    )
                    _WAITSPLIT[0] += 1
                    nop.engine = ins.engine
                    nop.sync_info = mybir.SyncInfo(
                        on_wait=extra[j : j + max_waits], on_update=[]
                    )
                    insns.insert(pos, nop)
                    pos += 1
                    i += 1
                si.on_wait = keep
            i += 1


def _make_masks():
    m = np.zeros((NP, 7 * NP), dtype=np.float32)
    for od in range(-3, 4):
        for p in range(NP):
            q = p - od
            if q // D == p // D and 0 <= q < NP:
                m[p, (od + 3) * NP + q] = 1.0
    return m.astype(ml_dtypes.bfloat16)


def _prep_inputs(x, guidance, convw, convb, ln_g, ln_b, w1, b1, w2, b2):
    f = np.float32
    w3 = np.ascontiguousarray(convw.reshape(NB, C, 27), dtype=f)
    cb = np.ascontiguousarray(convb, dtype=f)
    common = dict(
        masks=_make_masks(),
        w1t=np.ascontiguousarray(w1.T, dtype=f),
        b1=np.ascontiguousarray(b1, dtype=f),
        w2=np.ascontiguousarray(w2, dtype=f),
        b2=np.ascontiguousarray(b2, dtype=f),
        lng=np.ascontiguousarray(ln_g, dtype=f),
        lnb=np.ascontiguousarray(ln_b, dtype=f),
    )
    in_maps = []
    for core in range(N_CORES):
        b, hc = core // 2, core % 2
        ch0 = 48 * hc
        # xs[p=c*16+d, s*HW+j] = x[b, ch0+8s+c, d, j]
        arr = np.ascontiguousarray(x[b, ch0 : ch0 + 48], dtype=f)
        arr = arr.reshape(NSLAB, NCH, D, HW).transpose(1, 2, 0, 3).reshape(
            NP, NSLAB * HW
        )
        # cwx[p=c*16+d, s*81+t] = convw[br, ch0+8s+c, t27]  (d-independent)
        cw = w3[:, ch0 : ch0 + 48, :].reshape(NB, NSLAB, NCH, 27)
        cw = cw.transpose(2, 1, 0, 3).reshape(NCH, NSLAB * NB * 27)
        cwx = np.repeat(cw, D, axis=0)  # row c*16+d <- cw[c]
        cbs = cb[:, ch0 : ch0 + 48].reshape(NB, NSLAB, NCH)
        cbs = cbs.transpose(2, 1, 0).reshape(NCH, NSLAB * NB)
        cbx = np.repeat(cbs, D, axis=0)
        in_maps.append(
            dict(
                x=arr.astype(ml_dtypes.bfloat16),
                gd=np.ascontiguousarray(guidance[b], dtype=f),
                cwx=np.ascontiguousarray(cwx, dtype=f),
                cbx=np.ascontiguousarray(cbx, dtype=f),
                **common,
            )
        )
    return in_maps


_CACHED_NC = {}


def kernel(x, guidance, convw, convb, ln_g, ln_b, w1, b1, w2, b2):
    with_bias = bool(np.any(np.asarray(convb)))
    if with_bias not in _CACHED_NC:
        _CACHED_NC[with_bias] = _build_program(with_bias)
    nc = _CACHED_NC[with_bias]
    globals()["_LAST_NC"] = nc
    in_maps = _prep_inputs(
        x, guidance, convw, convb, ln_g, ln_b, w1, b1, w2, b2
    )
    res = run_bass_kernel_spmd(nc, in_maps, list(range(N_CORES)))
    out = np.empty((B, C, D, H, W), dtype=np.float32)
    for core in range(N_CORES):
        b, hc = core // 2, core % 2
        y = res.results[core]["y"].reshape(NCH, D, NSLAB, HW)
        out[b, 48 * hc : 48 * hc + 48] = (
            y.transpose(2, 0, 1, 3).reshape(48, D, H, W)
        )
    return out


if __name__ == "__main__":
    rng = np.random.default_rng(0)
    ins = dict(
        x=rng.standard_normal((B, C, D, H, W), dtype=np.float32),
        guidance=rng.standard_normal((B, G), dtype=np.float32),
        convw=(rng.standard_normal((NB, C, 1, K, K, K)) * 0.1).astype(np.float32),
        convb=np.zeros((NB, C), np.float32),
        ln_g=np.ones((C + G,), np.float32),
        ln_b=np.zeros((C + G,), np.float32),
        w1=(rng.standard_normal((C + G, HID)) * 0.05).astype(np.float32),
        b1=np.zeros((HID,), np.float32),
        w2=(rng.standard_normal((HID, NB)) * 0.05).astype(np.float32),
        b2=np.zeros((NB,), np.float32),
    )
    out = kernel(**ins)
    print("kernel ran, out shape", out.shape, "mean", float(np.abs(out).mean()))


# revision 13
# speedup vs baseline: 3.6515x; 1.0882x over previous
"""AttentionGuidedDynamicRangeDWConv3D on 8 Trainium2 NeuronCores.

Module: out = sum_i softmax(MLP(LN([mean_dhw(x), guidance])))[:, i]
                * dwconv3d(x, convw[i], convb[i], dil=i+1)
Shapes: x [4,96,16,56,56] f32, 3 branches of 3x3x3 depthwise conv with
dilations 1/2/3 ('same' zero padding).

Sharding: 8 cores = (batch b in 0..3) x (channel half hc in 0..1); each
core owns 48 channels of one batch at FULL depth.

Layout trick: partitions = (channel c in 0..8) x (depth d in 0..16), so a
single bf16 matmul with a 128x128 block-banded weight matrix applies an
entire depth-band of conv taps at once: out[(c,d), hw] +=
sum_od w[c, (od,oh,ow)] * x[(c,d+od), hw + oh*56+ow].  The 81 taps
(3 branches x 27) collapse into 25 matmul passes -- one per distinct
(oh,ow) pair -- accumulated in PSUM per 448-column (8 h-row) chunk.
Depth 'same' padding falls out of band truncation (no halo).  H/W 'same'
padding is exact via trimmed 2D access patterns (bf16 matmuls allow
strided APs; fp32r would not).

Band matrices are built by the Vector engine from host-supplied
shifted-identity masks scaled by per-partition weight columns.  The gate
MLP runs redundantly per core; the global mean-pool takes one pairwise
96x6-float AllGather (15us fixed latency in the cost model).  To hide
that latency plus the MLP, slabs 0 and 1 run UNGATED: their 27
single-branch passes accumulate the three branch convs into separate
PSUM banks, Act copies them to SBUF, and once the softmax weights are
ready the Vector engine does the weighted 3-way merge.  Slabs 2-5 use
gate-folded matrices (25 passes) and a plain Act PSUM->SBUF copy.
Engine streams are in-order, so emission order is chosen to keep DVE
(matrix builds) and Act (pool reductions + PSUM copies) ahead of the
Tensor engine throughout.
"""

import sys

if "/opt/trn_rl_repo" not in sys.path:
    sys.path.insert(0, "/opt/trn_rl_repo")

import ml_dtypes
import numpy as np

import concourse.bass as bass
import concourse.mybir as mybir
import concourse.tile as tile
from concourse.bass_utils import run_bass_kernel_spmd

F32 = mybir.dt.float32
BF16 = mybir.dt.bfloat16
ALU = mybir.AluOpType
ACTF = mybir.ActivationFunctionType

B, C, D, H, W = 4, 96, 16, 56, 56
G, HID, NB = 96, 24, 3
K = 3
DILS = (1, 2, 3)
LN_EPS = 1e-5
N_CORES = 8
NCH = 8                  # channels per slab
NSLAB = 6                # slabs per core (48 channels)
NUNG = 2                 # ungated slabs (hide the collective+MLP latency)
NP = NCH * D             # 128 partitions
HW = H * W               # 3136
CHUNK = 448              # 8 h-rows; PSUM bank-sized chunk
N_CHUNKS = 7
ROWS = CHUNK // W        # 8


def _pass_list(split_branches):
    """[(oh, ow, [(od, t), ...])].  split_branches: one pass per (branch,
    (oh,ow)) with the branch's (0,0) pass first (27 passes); else one pass
    per distinct (oh,ow) with (0,0) merged across branches first (25)."""
    out = []
    for i, d in enumerate(DILS):
        for oh in (0, -d, d):
            for ow in (0, -d, d) if oh == 0 else (-d, 0, d):
                if split_branches:
                    kh, kw = oh // d + 1, ow // d + 1
                    ents = [(kd * d - d, i * 27 + kd * 9 + kh * 3 + kw)
                            for kd in range(K)]
                    out.append((oh, ow, ents))
                else:
                    if (oh, ow) == (0, 0) and i > 0:
                        continue
                    ents = []
                    for j, dj in enumerate(DILS):
                        if oh in (-dj, 0, dj) and ow in (-dj, 0, dj):
                            kh, kw = oh // dj + 1, ow // dj + 1
                            ents += [(kd * dj - dj,
                                      j * 27 + kd * 9 + kh * 3 + kw)
                                     for kd in range(K)]
                    out.append((oh, ow, ents))
    if split_branches:
        assert len(out) == 27
    else:
        assert len(out) == 25
    assert sum(len(e) for _, _, e in out) == 81
    return out


def _build_program(with_bias):
    nc = bass.Bass()
    xin = nc.dram_tensor("x", [NP, NSLAB * HW], BF16, kind="ExternalInput")
    masks_in = nc.dram_tensor("masks", [NP, 7 * NP], BF16, kind="ExternalInput")
    cwx_in = nc.dram_tensor("cwx", [NP, NSLAB * 27 * NB], F32, kind="ExternalInput")
    cbx_in = nc.dram_tensor("cbx", [NP, NSLAB * NB], F32, kind="ExternalInput")
    gdin = nc.dram_tensor("gd", [G], F32, kind="ExternalInput")
    w1t_in = nc.dram_tensor("w1t", [HID, C + G], F32, kind="ExternalInput")
    b1_in = nc.dram_tensor("b1", [HID], F32, kind="ExternalInput")
    w2_in = nc.dram_tensor("w2", [HID, NB], F32, kind="ExternalInput")
    b2_in = nc.dram_tensor("b2", [NB], F32, kind="ExternalInput")
    lng_in = nc.dram_tensor("lng", [C + G], F32, kind="ExternalInput")
    lnb_in = nc.dram_tensor("lnb", [C + G], F32, kind="ExternalInput")
    yout = nc.dram_tensor("y", [NP, NSLAB * HW], F32, kind="ExternalOutput")

    p_ung = _pass_list(True)
    p_gat = _pass_list(False)

    with tile.TileContext(nc) as tc:
        with (
            tc.tile_pool(name="sbuf", bufs=1) as pool,
            tc.tile_pool(name="mats", bufs=2) as matpool,
            tc.tile_pool(name="outs", bufs=4) as outpool,
            tc.tile_pool(name="dram", bufs=1, space="DRAM") as dpool,
            tc.tile_pool(name="psum", bufs=1, space="PSUM") as ppool,
        ):
            xs = [
                pool.tile([NP, HW], BF16, tag=f"xs{s}", name=f"xs{s}")
                for s in range(NSLAB)
            ]
            masks = pool.tile([NP, 7 * NP], BF16, tag="masks")
            cwx = pool.tile([NP, NSLAB * 27 * NB], F32, tag="cwx")
            w_exp = pool.tile([NP, NSLAB * 27 * NB], F32, tag="w_exp")
            scr = pool.tile([NP, HW], BF16, tag="scr")
            part = pool.tile([NP, NSLAB], F32, tag="part")
            grow = pool.tile([1, 2 * NP * NSLAB], F32, tag="grow")
            bb = [
                [
                    pool.tile([NP, HW], F32, tag=f"bb{s}_{b}",
                              name=f"bb{s}_{b}")
                    for b in range(NB)
                ]
                for s in range(NUNG)
            ]
            g_row = pool.tile([1, C + G], F32, tag="g_row")
            gd_row = pool.tile([1, C + G], F32, tag="gd_row")
            lng = pool.tile([1, C + G], F32, tag="lng")
            lnb = pool.tile([1, C + G], F32, tag="lnb")
            gn_row = pool.tile([1, C + G], F32, tag="gn_row")
            gn_bc = pool.tile([HID, C + G], F32, tag="gn_bc")
            w1t = pool.tile([HID, C + G], F32, tag="w1t")
            prod = pool.tile([HID, C + G], F32, tag="prod")
            hvec = pool.tile([HID, 1], F32, tag="hvec")
            b1c = pool.tile([HID, 1], F32, tag="b1c")
            w2t = pool.tile([HID, NB], F32, tag="w2t")
            l2tmp = pool.tile([HID, NB], F32, tag="l2tmp")
            z72 = pool.tile([1, HID * NB], F32, tag="z72")
            zrow = pool.tile([1, NB], F32, tag="zrow")
            b2r = pool.tile([1, NB], F32, tag="b2r")
            wts = pool.tile([1, NB], F32, tag="wts")
            wts_bc = pool.tile([NP, NB], F32, tag="wts_bc")
            s1 = pool.tile([1, 1], F32, tag="s1")
            s2 = pool.tile([1, 1], F32, tag="s2")
            s3 = pool.tile([1, 1], F32, tag="s3")
            s4 = pool.tile([1, 1], F32, tag="s4")
            if with_bias:
                cbx = pool.tile([NP, NSLAB * NB], F32, tag="cbx")
                b_exp = pool.tile([NP, NSLAB], F32, tag="b_exp")
                betmp = pool.tile([NP, NSLAB * NB], F32, tag="betmp")

            cin = dpool.tile([NP, NSLAB], F32, tag="cin")
            cout = dpool.tile([2 * NP, NSLAB], F32, tag="cout")
            gb = dpool.tile([1, C + G], F32, tag="gb")
            wb = dpool.tile([1, NB], F32, tag="wb")

            v = nc.vector
            sc = nc.scalar

            # ---- A: loads (small weights first, then x slab-by-slab) ----
            nc.sync.dma_start(out=xs[0][:, :], in_=xin[:, 0:HW])
            nc.sync.dma_start(out=masks[:, :], in_=masks_in[:, :])
            nc.sync.dma_start(out=cwx[:, :], in_=cwx_in[:, :])
            for s in range(1, NSLAB):
                nc.sync.dma_start(
                    out=xs[s][:, :], in_=xin[:, s * HW : (s + 1) * HW]
                )
            nc.sync.dma_start(out=w1t[:, :], in_=w1t_in[:, :])
            nc.sync.dma_start(out=b1c[:, :], in_=b1_in[:, None])
            nc.sync.dma_start(out=w2t[:, :], in_=w2_in[:, :])
            nc.sync.dma_start(out=b2r[:, :], in_=b2_in[None, :])
            nc.sync.dma_start(out=lng[:, :], in_=lng_in[None, :])
            nc.sync.dma_start(out=lnb[:, :], in_=lnb_in[None, :])
            nc.sync.dma_start(out=g_row[:, C:], in_=gdin[None, :])
            if with_bias:
                nc.sync.dma_start(out=cbx[:, :], in_=cbx_in[:, :])

            # ---- B: per-slab plane sums on the Act engine ----
            for s in range(NSLAB):
                sc.activation(
                    scr[:, :], xs[s][:, :], ACTF.Copy,
                    accum_out=part[:, s : s + 1],
                )

            # ---- C: pairwise AllGather of raw plane sums ----
            nc.sync.dma_start(out=cin[:, :], in_=part[:, :])
            nc.gpsimd.collective_compute(
                "AllGather",
                ALU.bypass,
                replica_groups=[[2 * b, 2 * b + 1] for b in range(B)],
                ins=[cin.opt()],
                outs=[cout.opt()],
            )
            nc.sync.dma_start(out=grow[:, :], in_=cout[:, :])

            # ---- conv helpers ----
            def build_mats(s, passes, wsrc):
                mats = []
                for mi, (oh, ow, entries) in enumerate(passes):
                    mt = matpool.tile([NP, NP], BF16, tag=f"m{mi}")
                    for ei, (od, t) in enumerate(entries):
                        mk_in = masks[:, (od + 3) * NP : (od + 4) * NP]
                        wcol = wsrc[:, s * 81 + t : s * 81 + t + 1]
                        if ei == 0:
                            v.tensor_scalar(
                                out=mt[:, :], in0=mk_in, scalar1=wcol,
                                scalar2=None, op0=ALU.mult,
                            )
                        else:
                            v.scalar_tensor_tensor(
                                out=mt[:, :], in0=mk_in, scalar=wcol,
                                in1=mt[:, :], op0=ALU.mult, op1=ALU.add,
                            )
                    mats.append(mt)
                return mats

            def emit_pass(ps, pv, mt, xf, xv, ci, oh, ow, start, stop):
                if (oh, ow) == (0, 0):
                    nc.tensor.matmul(
                        ps[:, :], mt[:, :],
                        xf[:, ci * CHUNK : (ci + 1) * CHUNK],
                        start=start, stop=stop, skip_group_check=True,
                    )
                    return
                h0 = max(ci * ROWS, -oh if oh < 0 else 0)
                h1 = min(ci * ROWS + ROWS, H - (oh if oh > 0 else 0))
                if h1 <= h0:
                    return
                w0 = -ow if ow < 0 else 0
                w1 = W - (ow if ow > 0 else 0)
                nc.tensor.matmul(
                    pv[:, h0 - ci * ROWS : h1 - ci * ROWS, w0:w1],
                    mt[:, :],
                    xv[:, h0 + oh : h1 + oh, w0 + ow : w1 + ow],
                    start=start, stop=stop, skip_group_check=True,
                )

            def slab_views(s):
                xf = xs[s][:, :]
                return xf, xf.rearrange("p (h w) -> p h w", h=H, w=W)

            def emit_ungated_passmajor(s, mats):
                # branch-major / pass-major: PE consumes each matrix for 7
                # chunk-matmuls (1.3us) vs its 0.5us build -- no build-pacing
                # stalls on the very first slab.  Copies fire per branch.
                xf, xv = slab_views(s)
                for b in range(NB):
                    pss = [
                        ppool.tile([NP, CHUNK], F32, tag=f"ps{(3 * b + ci) % 8}",
                                   name=f"ups{s}_{b}_{ci}")
                        for ci in range(N_CHUNKS)
                    ]
                    pvs = [ps[:, :].rearrange("p (h w) -> p h w", h=ROWS, w=W)
                           for ps in pss]
                    for k in range(9):
                        oh, ow, _ = p_ung[b * 9 + k]
                        for ci in range(N_CHUNKS):
                            emit_pass(pss[ci], pvs[ci], mats[b * 9 + k], xf, xv,
                                      ci, oh, ow, k == 0, k == 8)
                    for ci in range(N_CHUNKS):
                        sc.activation(
                            bb[s][b][:, ci * CHUNK : (ci + 1) * CHUNK],
                            pss[ci][:, :], ACTF.Copy,
                        )

            def emit_ungated_chunkmajor(s, mats):
                xf, xv = slab_views(s)
                for ci in range(N_CHUNKS):
                    for b in range(NB):
                        ps = ppool.tile([NP, CHUNK], F32,
                                        tag=f"ps{(3 * ci + b) % 8}",
                                        name=f"ups{s}_{b}_{ci}")
                        pv = ps[:, :].rearrange("p (h w) -> p h w", h=ROWS, w=W)
                        for k in range(9):
                            oh, ow, _ = p_ung[b * 9 + k]
                            emit_pass(ps, pv, mats[b * 9 + k], xf, xv, ci,
                                      oh, ow, k == 0, k == 8)
                        sc.activation(
                            bb[s][b][:, ci * CHUNK : (ci + 1) * CHUNK],
                            ps[:, :], ACTF.Copy,
                        )

            def emit_merge(s):
                for ci in range(N_CHUNKS):
                    sl = slice(ci * CHUNK, (ci + 1) * CHUNK)
                    ot = outpool.tile([NP, CHUNK], F32, tag=f"o{ci % 4}")
                    v.tensor_scalar(
                        out=ot[:, :], in0=bb[s][0][:, sl],
                        scalar1=wts_bc[:, 0:1], scalar2=None, op0=ALU.mult,
                    )
                    for b in (1, 2):
                        v.scalar_tensor_tensor(
                            out=ot[:, :], in0=bb[s][b][:, sl],
                            scalar=wts_bc[:, b : b + 1], in1=ot[:, :],
                            op0=ALU.mult, op1=ALU.add,
                        )
                    if with_bias:
                        v.tensor_scalar(
                            out=ot[:, :], in0=ot[:, :],
                            scalar1=b_exp[:, s : s + 1], scalar2=None,
                            op0=ALU.add,
                        )
                    nc.sync.dma_start(
                        out=yout[:, s * HW + ci * CHUNK : s * HW + (ci + 1) * CHUNK],
                        in_=ot[:, :],
                    )

            def emit_gated_slab(s):
                mats = build_mats(s, p_gat, w_exp)
                xf, xv = slab_views(s)
                for ci in range(N_CHUNKS):
                    ps = ppool.tile([NP, CHUNK], F32, tag=f"ps{(s + ci) % 8}",
                                    name=f"gps{s}_{ci}")
                    pv = ps[:, :].rearrange("p (h w) -> p h w", h=ROWS, w=W)
                    for mi, (oh, ow, _) in enumerate(p_gat):
                        emit_pass(ps, pv, mats[mi], xf, xv, ci, oh, ow,
                                  mi == 0, mi == len(p_gat) - 1)
                    ot = outpool.tile([NP, CHUNK], F32, tag=f"o{ci % 4}")
                    sc.activation(ot[:, :], ps[:, :], ACTF.Copy)
                    if with_bias:
                        v.tensor_scalar(
                            out=ot[:, :], in0=ot[:, :],
                            scalar1=b_exp[:, s : s + 1], scalar2=None,
                            op0=ALU.add,
                        )
                    nc.sync.dma_start(
                        out=yout[:, s * HW + ci * CHUNK : s * HW + (ci + 1) * CHUNK],
                        in_=ot[:, :],
                    )

            # ---- D: slab 0 ungated (builds + matmuls + copies) ----
            mats0 = build_mats(0, p_ung, cwx)
            emit_ungated_passmajor(0, mats0)

            # ---- F1: slab 1 ungated ----
            mats1 = build_mats(1, p_ung, cwx)
            emit_ungated_chunkmajor(1, mats1)

            # ---- E: gate MLP ----
            # feat[48r + 8s + c] = sum_d cout[r, (c,d), s] / (D*HW)
            for r in range(2):
                gview = grow[:, r * NP * NSLAB : (r + 1) * NP * NSLAB].rearrange(
                    "a (c d s) -> a s c d", c=NCH, d=D, s=NSLAB
                )
                tview = g_row[:, 48 * r : 48 * r + 48].rearrange(
                    "a (s c) -> a s c", s=NSLAB, c=NCH
                )
                v.reduce_sum(tview, gview, axis=mybir.AxisListType.X)
            v.tensor_scalar_mul(g_row[:, 0:C], g_row[:, 0:C], 1.0 / (D * HW))

            # LayerNorm over 192 on one partition
            v.reduce_sum(s1[:, :], g_row[:, :], axis=mybir.AxisListType.X)
            v.tensor_scalar_mul(s1[:, :], s1[:, :], 1.0 / (C + G))  # mu
            v.tensor_scalar(
                out=gd_row[:, :], in0=g_row[:, :], scalar1=s1[:, :], scalar2=None,
                op0=ALU.subtract,
            )
            v.tensor_tensor(out=gn_row[:, :], in0=gd_row[:, :], in1=gd_row[:, :], op=ALU.mult)
            v.reduce_sum(s2[:, :], gn_row[:, :], axis=mybir.AxisListType.X)
            v.tensor_scalar(
                out=s2[:, :], in0=s2[:, :], scalar1=1.0 / (C + G), scalar2=LN_EPS,
                op0=ALU.mult, op1=ALU.add,
            )  # var + eps
            sc.activation(s3[:, :], s2[:, :], ACTF.Sqrt)
            # one Newton step for a clean sqrt
            v.reciprocal(s4[:, :], s3[:, :])
            v.tensor_tensor(out=s4[:, :], in0=s4[:, :], in1=s2[:, :], op=ALU.mult)
            v.tensor_tensor(out=s4[:, :], in0=s4[:, :], in1=s3[:, :], op=ALU.add)
            v.tensor_scalar_mul(s4[:, :], s4[:, :], 0.5)
            v.reciprocal(s3[:, :], s4[:, :])  # rstd
            v.tensor_scalar(
                out=gn_row[:, :], in0=gd_row[:, :], scalar1=s3[:, :], scalar2=None,
                op0=ALU.mult,
            )
            v.tensor_tensor(out=gn_row[:, :], in0=gn_row[:, :], in1=lng[:, :], op=ALU.mult)
            v.tensor_tensor(out=gn_row[:, :], in0=gn_row[:, :], in1=lnb[:, :], op=ALU.add)

            # MLP layer 1: h = gelu(gn @ w1 + b1) via row-products
            nc.sync.dma_start(out=gb[:, :], in_=gn_row[:, :])
            nc.sync.dma_start(out=gn_bc[:, :], in_=gb[:1, :].partition_broadcast(HID))
            v.tensor_tensor(out=prod[:, :], in0=w1t[:, :], in1=gn_bc[:, :], op=ALU.mult)
            v.reduce_sum(hvec[:, :], prod[:, :], axis=mybir.AxisListType.X)
            v.tensor_tensor(out=hvec[:, :], in0=hvec[:, :], in1=b1c[:, :], op=ALU.add)
            sc.activation(hvec[:, :], hvec[:, :], ACTF.Gelu)

            # MLP layer 2 via DRAM transpose bounce
            v.tensor_scalar(
                out=l2tmp[:, :], in0=w2t[:, :], scalar1=hvec[:, :], scalar2=None,
                op0=ALU.mult,
            )
            nc.sync.dma_start(out=z72[:, :], in_=l2tmp[:, :])
            z3 = z72[:, :].rearrange("a (j i) -> a j i", j=HID, i=NB)
            for i in range(NB):
                v.reduce_sum(zrow[:, i : i + 1], z3[:, :, i], axis=mybir.AxisListType.X)
            v.tensor_tensor(out=zrow[:, :], in0=zrow[:, :], in1=b2r[:, :], op=ALU.add)

            # softmax over 3
            v.reduce_max(s1[:, :], zrow[:, :], axis=mybir.AxisListType.X)
            v.tensor_scalar(
                out=zrow[:, :], in0=zrow[:, :], scalar1=s1[:, :], scalar2=None,
                op0=ALU.subtract,
            )
            sc.activation(zrow[:, :], zrow[:, :], ACTF.Exp)
            v.reduce_sum(s2[:, :], zrow[:, :], axis=mybir.AxisListType.X)
            v.reciprocal(s2[:, :], s2[:, :])
            v.tensor_scalar(
                out=wts[:, :], in0=zrow[:, :], scalar1=s2[:, :], scalar2=None,
                op0=ALU.mult,
            )

            # broadcast gate weights; fold into per-channel tap weights
            nc.sync.dma_start(out=wb[:, :], in_=wts[:, :])
            nc.sync.dma_start(out=wts_bc[:, :], in_=wb[:1, :].partition_broadcast(NP))
            for s in range(NUNG, NSLAB):
                for i in range(NB):
                    sl = slice(s * 81 + i * 27, s * 81 + (i + 1) * 27)
                    v.tensor_scalar(
                        out=w_exp[:, sl], in0=cwx[:, sl],
                        scalar1=wts_bc[:, i : i + 1], scalar2=None, op0=ALU.mult,
                    )
            if with_bias:
                for i in range(NB):
                    v.tensor_scalar(
                        out=betmp[:, i::NB], in0=cbx[:, i::NB],
                        scalar1=wts_bc[:, i : i + 1], scalar2=None, op0=ALU.mult,
                    )
                v.tensor_tensor(
                    out=b_exp[:, :], in0=betmp[:, 0::NB], in1=betmp[:, 1::NB],
                    op=ALU.add,
                )
                v.tensor_tensor(
                    out=b_exp[:, :], in0=b_exp[:, :], in1=betmp[:, 2::NB],
                    op=ALU.add,
                )

            # ---- G..J: gated slabs; merges (not PE-critical) trail ----
            emit_gated_slab(2)
            emit_merge(0)
            emit_gated_slab(3)
            emit_merge(1)
            for s in range(4, NSLAB):
                emit_gated_slab(s)

    _split_sem_waits(nc)
    return nc


_WAITSPLIT = [0]


def _split_sem_waits(nc, max_waits=1):
    """This walrus build rejects >1 SyncWait per instruction (and any wait on
    a Drain). Move excess waits onto same-engine NOPs inserted just before."""
    for bb in nc.main_func.blocks:
        insns = bb.instructions
        i = 0
        while i < len(insns):
            ins = insns[i]
            si = ins.sync_info
            limit = 0 if ins.opcode == "Drain" else max_waits
            if si is not None and si.on_wait is not None and len(si.on_wait) > limit:
                waits = list(si.on_wait)
                keep = waits[-limit:] if limit else []
                extra = waits[: len(waits) - limit]
                pos = i
                for j in range(0, len(extra), max_waits):
                    nop = mybir.InstNoOp(
                        name=f"I-waitsplit-{_WAITSPLIT[0]}", ins=[], outs=[]
                    )
                    _WAITSPLIT[0] += 1
                    nop.engine = ins.engine
                    nop.sync_info = mybir.SyncInfo(
                        on_wait=extra[j : j + max_waits], on_update=[]
                    )
                    insns.insert(pos, nop)
                    pos += 1
                    i += 1
                si.on_wait = keep
            i += 1


def _make_masks():
    m = np.zeros((NP, 7 * NP), dtype=np.float32)
    for od in range(-3, 4):
        for p in range(NP):
            q = p - od
            if q // D == p // D and 0 <= q < NP:
                m[p, (od + 3) * NP + q] = 1.0
    return m.astype(ml_dtypes.bfloat16)


def _prep_inputs(x, guidance, convw, convb, ln_g, ln_b, w1, b1, w2, b2):
    f = np.float32
    w3 = np.ascontiguousarray(convw.reshape(NB, C, 27), dtype=f)
    cb = np.ascontiguousarray(convb, dtype=f)
    common = dict(
        masks=_make_masks(),
        w1t=np.ascontiguousarray(w1.T, dtype=f),
        b1=np.ascontiguousarray(b1, dtype=f),
        w2=np.ascontiguousarray(w2, dtype=f),
        b2=np.ascontiguousarray(b2, dtype=f),
        lng=np.ascontiguousarray(ln_g, dtype=f),
        lnb=np.ascontiguousarray(ln_b, dtype=f),
    )
    in_maps = []
    for core in range(N_CORES):
        b, hc = core // 2, core % 2
        ch0 = 48 * hc
        # xs[p=c*16+d, s*HW+j] = x[b, ch0+8s+c, d, j]
        arr = np.ascontiguousarray(x[b, ch0 : ch0 + 48], dtype=f)
        arr = arr.reshape(NSLAB, NCH, D, HW).transpose(1, 2, 0, 3).reshape(
            NP, NSLAB * HW
        )
        # cwx[p=c*16+d, s*81+t] = convw[br, ch0+8s+c, t27]  (d-independent)
        cw = w3[:, ch0 : ch0 + 48, :].reshape(NB, NSLAB, NCH, 27)
        cw = cw.transpose(2, 1, 0, 3).reshape(NCH, NSLAB * NB * 27)
        cwx = np.repeat(cw, D, axis=0)  # row c*16+d <- cw[c]
        cbs = cb[:, ch0 : ch0 + 48].reshape(NB, NSLAB, NCH)
        cbs = cbs.transpose(2, 1, 0).reshape(NCH, NSLAB * NB)
        cbx = np.repeat(cbs, D, axis=0)
        in_maps.append(
            dict(
                x=arr.astype(ml_dtypes.bfloat16),
                gd=np.ascontiguousarray(guidance[b], dtype=f),
                cwx=np.ascontiguousarray(cwx, dtype=f),
                cbx=np.ascontiguousarray(cbx, dtype=f),
                **common,
            )
        )
    return in_maps


_CACHED_NC = {}


def kernel(x, guidance, convw, convb, ln_g, ln_b, w1, b1, w2, b2):
    with_bias = bool(np.any(np.asarray(convb)))
    if with_bias not in _CACHED_NC:
        _CACHED_NC[with_bias] = _build_program(with_bias)
    nc = _CACHED_NC[with_bias]
    globals()["_LAST_NC"] = nc
    in_maps = _prep_inputs(
        x, guidance, convw, convb, ln_g, ln_b, w1, b1, w2, b2
    )
    res = run_bass_kernel_spmd(nc, in_maps, list(range(N_CORES)))
    out = np.empty((B, C, D, H, W), dtype=np.float32)
    for core in range(N_CORES):
        b, hc = core // 2, core % 2
        y = res.results[core]["y"].reshape(NCH, D, NSLAB, HW)
        out[b, 48 * hc : 48 * hc + 48] = (
            y.transpose(2, 0, 1, 3).reshape(48, D, H, W)
        )
    return out


if __name__ == "__main__":
    rng = np.random.default_rng(0)
    ins = dict(
        x=rng.standard_normal((B, C, D, H, W), dtype=np.float32),
        guidance=rng.standard_normal((B, G), dtype=np.float32),
        convw=(rng.standard_normal((NB, C, 1, K, K, K)) * 0.1).astype(np.float32),
        convb=np.zeros((NB, C), np.float32),
        ln_g=np.ones((C + G,), np.float32),
        ln_b=np.zeros((C + G,), np.float32),
        w1=(rng.standard_normal((C + G, HID)) * 0.05).astype(np.float32),
        b1=np.zeros((HID,), np.float32),
        w2=(rng.standard_normal((HID, NB)) * 0.05).astype(np.float32),
        b2=np.zeros((NB,), np.float32),
    )
    out = kernel(**ins)
    print("kernel ran, out shape", out.shape, "mean", float(np.abs(out).mean()))


# revision 19
# speedup vs baseline: 3.7659x; 1.0313x over previous
"""AttentionGuidedDynamicRangeDWConv3D on 8 Trainium2 NeuronCores.

Module: out = sum_i softmax(MLP(LN([mean_dhw(x), guidance])))[:, i]
                * dwconv3d(x, convw[i], convb[i], dil=i+1)
Shapes: x [4,96,16,56,56] f32, 3 branches of 3x3x3 depthwise conv with
dilations 1/2/3 ('same' zero padding).

Sharding: 8 cores = (batch b in 0..3) x (channel half hc in 0..1); each
core owns 48 channels of one batch at FULL depth.

Layout trick: partitions = (channel c in 0..8) x (depth d in 0..16), so a
single bf16 matmul with a 128x128 block-banded weight matrix applies an
entire depth-band of conv taps at once: out[(c,d), hw] +=
sum_od w[c, (od,oh,ow)] * x[(c,d+od), hw + oh*56+ow].  The 81 taps
(3 branches x 27) collapse into 25 matmul passes -- one per distinct
(oh,ow) pair -- accumulated in PSUM per 448-column (8 h-row) chunk.
Depth 'same' padding falls out of band truncation (no halo).  H/W 'same'
padding is exact via trimmed 2D access patterns (bf16 matmuls allow
strided APs; fp32r would not).

Band matrices are built by the Vector engine from host-supplied
shifted-identity masks scaled by per-partition weight columns.  The gate
MLP runs redundantly per core; the global mean-pool takes one pairwise
96x6-float AllGather (15us fixed latency in the cost model).  To hide
that latency plus the MLP, slabs 0 and 1 run UNGATED: their 27
single-branch passes accumulate the three branch convs into separate
PSUM banks, Act copies them to SBUF, and once the softmax weights are
ready the Vector engine does the weighted 3-way merge.  Slabs 2-5 use
gate-folded matrices (25 passes) and a plain Act PSUM->SBUF copy.
Engine streams are in-order, so emission order is chosen to keep DVE
(matrix builds) and Act (pool reductions + PSUM copies) ahead of the
Tensor engine throughout.
"""

import sys

if "/opt/trn_rl_repo" not in sys.path:
    sys.path.insert(0, "/opt/trn_rl_repo")

import ml_dtypes
import numpy as np

import concourse.bass as bass
import concourse.mybir as mybir
import concourse.tile as tile
from concourse.bass_utils import run_bass_kernel_spmd

F32 = mybir.dt.float32
BF16 = mybir.dt.bfloat16
ALU = mybir.AluOpType
ACTF = mybir.ActivationFunctionType

B, C, D, H, W = 4, 96, 16, 56, 56
G, HID, NB = 96, 24, 3
K = 3
DILS = (1, 2, 3)
LN_EPS = 1e-5
N_CORES = 8
NCH = 8                  # channels per slab
NSLAB = 6                # slabs per core (48 channels)
NUNG = 2                 # ungated slabs (hide the collective+MLP latency)
NP = NCH * D             # 128 partitions
HW = H * W               # 3136
CHUNK = 448              # 8 h-rows; PSUM bank-sized chunk
N_CHUNKS = 7
ROWS = CHUNK // W        # 8


def _pass_list(split_branches):
    """[(oh, ow, [(od, t), ...])].  split_branches: one pass per (branch,
    (oh,ow)) with the branch's (0,0) pass first (27 passes); else one pass
    per distinct (oh,ow) with (0,0) merged across branches first (25)."""
    out = []
    for i, d in enumerate(DILS):
        for oh in (0, -d, d):
            for ow in (0, -d, d) if oh == 0 else (-d, 0, d):
                if split_branches:
                    kh, kw = oh // d + 1, ow // d + 1
                    ents = [(kd * d - d, i * 27 + kd * 9 + kh * 3 + kw)
                            for kd in range(K)]
                    out.append((oh, ow, ents))
                else:
                    if (oh, ow) == (0, 0) and i > 0:
                        continue
                    ents = []
                    for j, dj in enumerate(DILS):
                        if oh in (-dj, 0, dj) and ow in (-dj, 0, dj):
                            kh, kw = oh // dj + 1, ow // dj + 1
                            ents += [(kd * dj - dj,
                                      j * 27 + kd * 9 + kh * 3 + kw)
                                     for kd in range(K)]
                    out.append((oh, ow, ents))
    if split_branches:
        assert len(out) == 27
    else:
        assert len(out) == 25
    assert sum(len(e) for _, _, e in out) == 81
    return out


def _build_program(with_bias):
    nc = bass.Bass()
    xin = nc.dram_tensor("x", [NP, NSLAB * HW], BF16, kind="ExternalInput")
    masks_in = nc.dram_tensor("masks", [NP, 7 * NP], BF16, kind="ExternalInput")
    cwx_in = nc.dram_tensor("cwx", [NP, NSLAB * 27 * NB], F32, kind="ExternalInput")
    cbx_in = nc.dram_tensor("cbx", [NP, NSLAB * NB], F32, kind="ExternalInput")
    gdin = nc.dram_tensor("gd", [G], F32, kind="ExternalInput")
    w1t_in = nc.dram_tensor("w1t", [HID, C + G], F32, kind="ExternalInput")
    b1_in = nc.dram_tensor("b1", [HID], F32, kind="ExternalInput")
    w2_in = nc.dram_tensor("w2", [HID, NB], F32, kind="ExternalInput")
    b2_in = nc.dram_tensor("b2", [NB], F32, kind="ExternalInput")
    lng_in = nc.dram_tensor("lng", [C + G], F32, kind="ExternalInput")
    lnb_in = nc.dram_tensor("lnb", [C + G], F32, kind="ExternalInput")
    yout = nc.dram_tensor("y", [NP, NSLAB * HW], F32, kind="ExternalOutput")

    p_ung = _pass_list(True)
    p_gat = _pass_list(False)

    with tile.TileContext(nc) as tc:
        with (
            tc.tile_pool(name="sbuf", bufs=1) as pool,
            tc.tile_pool(name="mats", bufs=2) as matpool,
            tc.tile_pool(name="outs", bufs=4) as outpool,
            tc.tile_pool(name="dram", bufs=1, space="DRAM") as dpool,
            tc.tile_pool(name="psum", bufs=1, space="PSUM") as ppool,
        ):
            xs = [
                pool.tile([NP, HW], BF16, tag=f"xs{s}", name=f"xs{s}")
                for s in range(NSLAB)
            ]
            masks = pool.tile([NP, 7 * NP], BF16, tag="masks")
            cwx = pool.tile([NP, NSLAB * 27 * NB], F32, tag="cwx")
            w_exp = pool.tile([NP, NSLAB * 27 * NB], F32, tag="w_exp")
            scr = pool.tile([NP, HW], BF16, tag="scr")
            part = pool.tile([NP, NSLAB], F32, tag="part")
            grow = pool.tile([1, 2 * NP * NSLAB], F32, tag="grow")
            bb = [
                [
                    pool.tile([NP, HW], F32, tag=f"bb{s}_{b}",
                              name=f"bb{s}_{b}")
                    for b in range(NB)
                ]
                for s in range(NUNG)
            ]
            g_row = pool.tile([1, C + G], F32, tag="g_row")
            gd_row = pool.tile([1, C + G], F32, tag="gd_row")
            lng = pool.tile([1, C + G], F32, tag="lng")
            lnb = pool.tile([1, C + G], F32, tag="lnb")
            gn_row = pool.tile([1, C + G], F32, tag="gn_row")
            gn_bc = pool.tile([HID, C + G], F32, tag="gn_bc")
            w1t = pool.tile([HID, C + G], F32, tag="w1t")
            prod = pool.tile([HID, C + G], F32, tag="prod")
            hvec = pool.tile([HID, 1], F32, tag="hvec")
            b1c = pool.tile([HID, 1], F32, tag="b1c")
            w2t = pool.tile([HID, NB], F32, tag="w2t")
            l2tmp = pool.tile([HID, NB], F32, tag="l2tmp")
            z72 = pool.tile([1, HID * NB], F32, tag="z72")
            zrow = pool.tile([1, NB], F32, tag="zrow")
            b2r = pool.tile([1, NB], F32, tag="b2r")
            wts = pool.tile([1, NB], F32, tag="wts")
            wts_bc = pool.tile([NP, NB], F32, tag="wts_bc")
            s1 = pool.tile([1, 1], F32, tag="s1")
            s2 = pool.tile([1, 1], F32, tag="s2")
            s3 = pool.tile([1, 1], F32, tag="s3")
            s4 = pool.tile([1, 1], F32, tag="s4")
            if with_bias:
                cbx = pool.tile([NP, NSLAB * NB], F32, tag="cbx")
                b_exp = pool.tile([NP, NSLAB], F32, tag="b_exp")
                betmp = pool.tile([NP, NSLAB * NB], F32, tag="betmp")

            cin = dpool.tile([NP, NSLAB], F32, tag="cin")
            cout = dpool.tile([2 * NP, NSLAB], F32, tag="cout")
            gb = dpool.tile([1, C + G], F32, tag="gb")
            wb = dpool.tile([1, NB], F32, tag="wb")

            v = nc.vector
            sc = nc.scalar

            # ---- A: loads (small weights first, then x slab-by-slab) ----
            nc.sync.dma_start(out=masks[:, :], in_=masks_in[:, :])
            nc.sync.dma_start(out=cwx[:, :], in_=cwx_in[:, :])
            nc.sync.dma_start(out=xs[0][:, : HW // 2], in_=xin[:, : HW // 2])
            nc.sync.dma_start(out=xs[0][:, HW // 2 :], in_=xin[:, HW // 2 : HW])
            for s in range(1, NSLAB):
                nc.sync.dma_start(
                    out=xs[s][:, :], in_=xin[:, s * HW : (s + 1) * HW]
                )
            nc.sync.dma_start(out=w1t[:, :], in_=w1t_in[:, :])
            nc.sync.dma_start(out=b1c[:, :], in_=b1_in[:, None])
            nc.sync.dma_start(out=w2t[:, :], in_=w2_in[:, :])
            nc.sync.dma_start(out=b2r[:, :], in_=b2_in[None, :])
            nc.sync.dma_start(out=lng[:, :], in_=lng_in[None, :])
            nc.sync.dma_start(out=lnb[:, :], in_=lnb_in[None, :])
            nc.sync.dma_start(out=g_row[:, C:], in_=gdin[None, :])
            if with_bias:
                nc.sync.dma_start(out=cbx[:, :], in_=cbx_in[:, :])

            # ---- B: plane sums: slabs 0-3 on Act now; slab 4 threaded
            # between slab-0 branch copies; slab 5 on DVE (Act stays just
            # ahead of both the PSUM-copy demand and the collective) ----
            for s in range(NSLAB - 2):
                sc.activation(
                    scr[:, :], xs[s][:, :], ACTF.Copy,
                    accum_out=part[:, s : s + 1],
                )

            # ---- C: pairwise AllGather of raw plane sums ----
            nc.sync.dma_start(out=cin[:, :], in_=part[:, :])
            nc.gpsimd.collective_compute(
                "AllGather",
                ALU.bypass,
                replica_groups=[[2 * b, 2 * b + 1] for b in range(B)],
                ins=[cin.opt()],
                outs=[cout.opt()],
            )
            nc.sync.dma_start(out=grow[:, :], in_=cout[:, :])

            # ---- conv helpers ----
            def build_mats(s, passes, wsrc):
                mats = []
                for mi, (oh, ow, entries) in enumerate(passes):
                    mt = matpool.tile([NP, NP], BF16, tag=f"m{mi}")
                    for ei, (od, t) in enumerate(entries):
                        mk_in = masks[:, (od + 3) * NP : (od + 4) * NP]
                        wcol = wsrc[:, s * 81 + t : s * 81 + t + 1]
                        if ei == 0:
                            v.tensor_scalar(
                                out=mt[:, :], in0=mk_in, scalar1=wcol,
                                scalar2=None, op0=ALU.mult,
                            )
                        else:
                            v.scalar_tensor_tensor(
                                out=mt[:, :], in0=mk_in, scalar=wcol,
                                in1=mt[:, :], op0=ALU.mult, op1=ALU.add,
                            )
                    mats.append(mt)
                return mats

            def emit_pass(ps, pv, mt, xf, xv, ci, oh, ow, start, stop):
                if (oh, ow) == (0, 0):
                    nc.tensor.matmul(
                        ps[:, :], mt[:, :],
                        xf[:, ci * CHUNK : (ci + 1) * CHUNK],
                        start=start, stop=stop, skip_group_check=True,
                    )
                    return
                h0 = max(ci * ROWS, -oh if oh < 0 else 0)
                h1 = min(ci * ROWS + ROWS, H - (oh if oh > 0 else 0))
                if h1 <= h0:
                    return
                w0 = -ow if ow < 0 else 0
                w1 = W - (ow if ow > 0 else 0)
                nc.tensor.matmul(
                    pv[:, h0 - ci * ROWS : h1 - ci * ROWS, w0:w1],
                    mt[:, :],
                    xv[:, h0 + oh : h1 + oh, w0 + ow : w1 + ow],
                    start=start, stop=stop, skip_group_check=True,
                )

            def slab_views(s):
                xf = xs[s][:, :]
                return xf, xf.rearrange("p (h w) -> p h w", h=H, w=W)

            def emit_ungated_passmajor(s, mats, after_branch=None):
                # branch-major / pass-major: PE consumes each matrix for 7
                # chunk-matmuls (1.3us) vs its 0.5us build -- no build-pacing
                # stalls on the very first slab.  Copies fire per branch.
                xf, xv = slab_views(s)
                for b in range(NB):
                    pss = [
                        ppool.tile([NP, CHUNK], F32, tag=f"ps{(3 * b + ci) % 8}",
                                   name=f"ups{s}_{b}_{ci}")
                        for ci in range(N_CHUNKS)
                    ]
                    pvs = [ps[:, :].rearrange("p (h w) -> p h w", h=ROWS, w=W)
                           for ps in pss]
                    for k in range(9):
                        oh, ow, _ = p_ung[b * 9 + k]
                        for ci in range(N_CHUNKS):
                            emit_pass(pss[ci], pvs[ci], mats[b * 9 + k], xf, xv,
                                      ci, oh, ow, k == 0, k == 8)
                    # copy order matches the tag order the NEXT consumer
                    # (branch b+1 / the following slab) acquires, so its
                    # start=True matmuls unblock as early as possible
                    for ci in range(N_CHUNKS):
                        sc.activation(
                            bb[s][b][:, ci * CHUNK : (ci + 1) * CHUNK],
                            pss[ci][:, :], ACTF.Copy,
                        )
                    if after_branch and b in after_branch:
                        after_branch[b]()

            def emit_ungated_chunkmajor(s, mats):
                xf, xv = slab_views(s)
                for ci in range(N_CHUNKS):
                    for b in range(NB):
                        ps = ppool.tile([NP, CHUNK], F32,
                                        tag=f"ps{(3 * ci + b) % 8}",
                                        name=f"ups{s}_{b}_{ci}")
                        pv = ps[:, :].rearrange("p (h w) -> p h w", h=ROWS, w=W)
                        for k in range(9):
                            oh, ow, _ = p_ung[b * 9 + k]
                            emit_pass(ps, pv, mats[b * 9 + k], xf, xv, ci,
                                      oh, ow, k == 0, k == 8)
                        sc.activation(
                            bb[s][b][:, ci * CHUNK : (ci + 1) * CHUNK],
                            ps[:, :], ACTF.Copy,
                        )

            def emit_merge(s):
                for ci in range(N_CHUNKS):
                    sl = slice(ci * CHUNK, (ci + 1) * CHUNK)
                    ot = outpool.tile([NP, CHUNK], F32, tag=f"o{ci % 4}")
                    v.tensor_scalar(
                        out=ot[:, :], in0=bb[s][0][:, sl],
                        scalar1=wts_bc[:, 0:1], scalar2=None, op0=ALU.mult,
                    )
                    for b in (1, 2):
                        v.scalar_tensor_tensor(
                            out=ot[:, :], in0=bb[s][b][:, sl],
                            scalar=wts_bc[:, b : b + 1], in1=ot[:, :],
                            op0=ALU.mult, op1=ALU.add,
                        )
                    if with_bias:
                        v.tensor_scalar(
                            out=ot[:, :], in0=ot[:, :],
                            scalar1=b_exp[:, s : s + 1], scalar2=None,
                            op0=ALU.add,
                        )
                    nc.sync.dma_start(
                        out=yout[:, s * HW + ci * CHUNK : s * HW + (ci + 1) * CHUNK],
                        in_=ot[:, :],
                    )

            def emit_gated_out(s, ci, ps):
                ot = outpool.tile([NP, CHUNK], F32, tag=f"o{ci % 4}",
                                  name=f"ot{s}_{ci}")
                sc.activation(ot[:, :], ps[:, :], ACTF.Copy)
                if with_bias:
                    v.tensor_scalar(
                        out=ot[:, :], in0=ot[:, :],
                        scalar1=b_exp[:, s : s + 1], scalar2=None,
                        op0=ALU.add,
                    )
                nc.sync.dma_start(
                    out=yout[:, s * HW + ci * CHUNK : s * HW + (ci + 1) * CHUNK],
                    in_=ot[:, :],
                )

            def emit_gated_slab(s, passmajor=False):
                mats = build_mats(s, p_gat, w_exp)
                xf, xv = slab_views(s)
                if passmajor:
                    # consume each matrix for 7 chunk-matmuls: tolerates
                    # just-in-time builds right after the gate MLP lands
                    pss = [
                        ppool.tile([NP, CHUNK], F32, tag=f"ps{ci}",
                                   name=f"gps{s}_{ci}")
                        for ci in range(N_CHUNKS)
                    ]
                    pvs = [ps[:, :].rearrange("p (h w) -> p h w", h=ROWS, w=W)
                           for ps in pss]
                    for mi, (oh, ow, _) in enumerate(p_gat):
                        for ci in range(N_CHUNKS):
                            emit_pass(pss[ci], pvs[ci], mats[mi], xf, xv, ci,
                                      oh, ow, mi == 0, mi == len(p_gat) - 1)
                    for ci in [1, 2, 3, 4, 5, 6, 0]:
                        emit_gated_out(s, ci, pss[ci])
                    return
                for ci in range(N_CHUNKS):
                    ps = ppool.tile([NP, CHUNK], F32, tag=f"ps{ci}",
                                    name=f"gps{s}_{ci}")
                    pv = ps[:, :].rearrange("p (h w) -> p h w", h=ROWS, w=W)
                    for mi, (oh, ow, _) in enumerate(p_gat):
                        emit_pass(ps, pv, mats[mi], xf, xv, ci, oh, ow,
                                  mi == 0, mi == len(p_gat) - 1)
                    emit_gated_out(s, ci, ps)

            # ---- D: slab 0 ungated (builds + matmuls + copies) ----
            def red4():
                sc.activation(
                    scr[:, :], xs[4][:, :], ACTF.Copy,
                    accum_out=part[:, 4:5],
                )

            mats0 = build_mats(0, p_ung, cwx)
            emit_ungated_chunkmajor(0, mats0)
            red4()

            # slab-5 plane sum on DVE (Act is busy; DVE has a lull here)
            v.reduce_sum(
                part[:, NSLAB - 1 : NSLAB], xs[NSLAB - 1][:, :],
                axis=mybir.AxisListType.X,
            )

            # ---- F1: slab 1 ungated ----
            mats1 = build_mats(1, p_ung, cwx)
            emit_ungated_chunkmajor(1, mats1)

            # ---- E: gate MLP ----
            # feat[48r + 8s + c] = sum_d cout[r, (c,d), s] / (D*HW)
            for r in range(2):
                gview = grow[:, r * NP * NSLAB : (r + 1) * NP * NSLAB].rearrange(
                    "a (c d s) -> a s c d", c=NCH, d=D, s=NSLAB
                )
                tview = g_row[:, 48 * r : 48 * r + 48].rearrange(
                    "a (s c) -> a s c", s=NSLAB, c=NCH
                )
                v.reduce_sum(tview, gview, axis=mybir.AxisListType.X)
            v.tensor_scalar_mul(g_row[:, 0:C], g_row[:, 0:C], 1.0 / (D * HW))

            # LayerNorm over 192 on one partition
            v.reduce_sum(s1[:, :], g_row[:, :], axis=mybir.AxisListType.X)
            v.tensor_scalar_mul(s1[:, :], s1[:, :], 1.0 / (C + G))  # mu
            v.tensor_scalar(
                out=gd_row[:, :], in0=g_row[:, :], scalar1=s1[:, :], scalar2=None,
                op0=ALU.subtract,
            )
            v.tensor_tensor(out=gn_row[:, :], in0=gd_row[:, :], in1=gd_row[:, :], op=ALU.mult)
            v.reduce_sum(s2[:, :], gn_row[:, :], axis=mybir.AxisListType.X)
            v.tensor_scalar(
                out=s2[:, :], in0=s2[:, :], scalar1=1.0 / (C + G), scalar2=LN_EPS,
                op0=ALU.mult, op1=ALU.add,
            )  # var + eps
            sc.activation(s3[:, :], s2[:, :], ACTF.Sqrt)
            # one Newton step for a clean sqrt
            v.reciprocal(s4[:, :], s3[:, :])
            v.tensor_tensor(out=s4[:, :], in0=s4[:, :], in1=s2[:, :], op=ALU.mult)
            v.tensor_tensor(out=s4[:, :], in0=s4[:, :], in1=s3[:, :], op=ALU.add)
            v.tensor_scalar_mul(s4[:, :], s4[:, :], 0.5)
            v.reciprocal(s3[:, :], s4[:, :])  # rstd
            v.tensor_scalar(
                out=gn_row[:, :], in0=gd_row[:, :], scalar1=s3[:, :], scalar2=None,
                op0=ALU.mult,
            )
            v.tensor_tensor(out=gn_row[:, :], in0=gn_row[:, :], in1=lng[:, :], op=ALU.mult)
            v.tensor_tensor(out=gn_row[:, :], in0=gn_row[:, :], in1=lnb[:, :], op=ALU.add)

            # MLP layer 1: h = gelu(gn @ w1 + b1) via row-products
            nc.sync.dma_start(out=gb[:, :], in_=gn_row[:, :])
            nc.sync.dma_start(out=gn_bc[:, :], in_=gb[:1, :].partition_broadcast(HID))
            v.tensor_tensor(out=prod[:, :], in0=w1t[:, :], in1=gn_bc[:, :], op=ALU.mult)
            v.reduce_sum(hvec[:, :], prod[:, :], axis=mybir.AxisListType.X)
            v.tensor_tensor(out=hvec[:, :], in0=hvec[:, :], in1=b1c[:, :], op=ALU.add)
            sc.activation(hvec[:, :], hvec[:, :], ACTF.Gelu)

            # MLP layer 2 via DRAM transpose bounce
            v.tensor_scalar(
                out=l2tmp[:, :], in0=w2t[:, :], scalar1=hvec[:, :], scalar2=None,
                op0=ALU.mult,
            )
            nc.sync.dma_start(out=z72[:, :], in_=l2tmp[:, :])
            z3 = z72[:, :].rearrange("a (j i) -> a j i", j=HID, i=NB)
            for i in range(NB):
                v.reduce_sum(zrow[:, i : i + 1], z3[:, :, i], axis=mybir.AxisListType.X)
            v.tensor_tensor(out=zrow[:, :], in0=zrow[:, :], in1=b2r[:, :], op=ALU.add)

            # softmax over 3
            v.reduce_max(s1[:, :], zrow[:, :], axis=mybir.AxisListType.X)
            v.tensor_scalar(
                out=zrow[:, :], in0=zrow[:, :], scalar1=s1[:, :], scalar2=None,
                op0=ALU.subtract,
            )
            sc.activation(zrow[:, :], zrow[:, :], ACTF.Exp)
            v.reduce_sum(s2[:, :], zrow[:, :], axis=mybir.AxisListType.X)
            v.reciprocal(s2[:, :], s2[:, :])
            v.tensor_scalar(
                out=wts[:, :], in0=zrow[:, :], scalar1=s2[:, :], scalar2=None,
                op0=ALU.mult,
            )

            # broadcast gate weights; fold into per-channel tap weights
            nc.sync.dma_start(out=wb[:, :], in_=wts[:, :])
            nc.sync.dma_start(out=wts_bc[:, :], in_=wb[:1, :].partition_broadcast(NP))
            for s in range(NUNG, NSLAB):
                for i in range(NB):
                    sl = slice(s * 81 + i * 27, s * 81 + (i + 1) * 27)
                    v.tensor_scalar(
                        out=w_exp[:, sl], in0=cwx[:, sl],
                        scalar1=wts_bc[:, i : i + 1], scalar2=None, op0=ALU.mult,
                    )
            if with_bias:
                for i in range(NB):
                    v.tensor_scalar(
                        out=betmp[:, i::NB], in0=cbx[:, i::NB],
                        scalar1=wts_bc[:, i : i + 1], scalar2=None, op0=ALU.mult,
                    )
                v.tensor_tensor(
                    out=b_exp[:, :], in0=betmp[:, 0::NB], in1=betmp[:, 1::NB],
                    op=ALU.add,
                )
                v.tensor_tensor(
                    out=b_exp[:, :], in0=b_exp[:, :], in1=betmp[:, 2::NB],
                    op=ALU.add,
                )

            # ---- G..J: gated slabs; merges (not PE-critical) trail ----
            emit_gated_slab(2)
            emit_merge(0)
            emit_gated_slab(3)
            emit_merge(1)
            for s in range(4, NSLAB):
                emit_gated_slab(s)

    _split_sem_waits(nc)
    return nc


_WAITSPLIT = [0]


def _split_sem_waits(nc, max_waits=1):
    """This walrus build rejects >1 SyncWait per instruction (and any wait on
    a Drain). Move excess waits onto same-engine NOPs inserted just before."""
    for bb in nc.main_func.blocks:
        insns = bb.instructions
        i = 0
        while i < len(insns):
            ins = insns[i]
            si = ins.sync_info
            limit = 0 if ins.opcode == "Drain" else max_waits
            if si is not None and si.on_wait is not None and len(si.on_wait) > limit:
                waits = list(si.on_wait)
                keep = waits[-limit:] if limit else []
                extra = waits[: len(waits) - limit]
                pos = i
                for j in range(0, len(extra), max_waits):
                    nop = mybir.InstNoOp(
                        name=f"I-waitsplit-{_WAITSPLIT[0]}", ins=[], outs=[]
                    )
                    _WAITSPLIT[0] += 1
                    nop.engine = ins.engine
                    nop.sync_info = mybir.SyncInfo(
                        on_wait=extra[j : j + max_waits], on_update=[]
                    )
                    insns.insert(pos, nop)
                    pos += 1
                    i += 1
                si.on_wait = keep
            i += 1


def _make_masks():
    m = np.zeros((NP, 7 * NP), dtype=np.float32)
    for od in range(-3, 4):
        for p in range(NP):
            q = p - od
            if q // D == p // D and 0 <= q < NP:
                m[p, (od + 3) * NP + q] = 1.0
    return m.astype(ml_dtypes.bfloat16)


def _prep_inputs(x, guidance, convw, convb, ln_g, ln_b, w1, b1, w2, b2):
    f = np.float32
    w3 = np.ascontiguousarray(convw.reshape(NB, C, 27), dtype=f)
    cb = np.ascontiguousarray(convb, dtype=f)
    common = dict(
        masks=_make_masks(),
        w1t=np.ascontiguousarray(w1.T, dtype=f),
        b1=np.ascontiguousarray(b1, dtype=f),
        w2=np.ascontiguousarray(w2, dtype=f),
        b2=np.ascontiguousarray(b2, dtype=f),
        lng=np.ascontiguousarray(ln_g, dtype=f),
        lnb=np.ascontiguousarray(ln_b, dtype=f),
    )
    in_maps = []
    for core in range(N_CORES):
        b, hc = core // 2, core % 2
        ch0 = 48 * hc
        # xs[p=c*16+d, s*HW+j] = x[b, ch0+8s+c, d, j]
        arr = np.ascontiguousarray(x[b, ch0 : ch0 + 48], dtype=f)
        arr = arr.reshape(NSLAB, NCH, D, HW).transpose(1, 2, 0, 3).reshape(
            NP, NSLAB * HW
        )
        # cwx[p=c*16+d, s*81+t] = convw[br, ch0+8s+c, t27]  (d-independent)
        cw = w3[:, ch0 : ch0 + 48, :].reshape(NB, NSLAB, NCH, 27)
        cw = cw.transpose(2, 1, 0, 3).reshape(NCH, NSLAB * NB * 27)
        cwx = np.repeat(cw, D, axis=0)  # row c*16+d <- cw[c]
        cbs = cb[:, ch0 : ch0 + 48].reshape(NB, NSLAB, NCH)
        cbs = cbs.transpose(2, 1, 0).reshape(NCH, NSLAB * NB)
        cbx = np.repeat(cbs, D, axis=0)
        in_maps.append(
            dict(
                x=arr.astype(ml_dtypes.bfloat16),
                gd=np.ascontiguousarray(guidance[b], dtype=f),
                cwx=np.ascontiguousarray(cwx, dtype=f),
                cbx=np.ascontiguousarray(cbx, dtype=f),
                **common,
            )
        )
    return in_maps


_CACHED_NC = {}


def kernel(x, guidance, convw, convb, ln_g, ln_b, w1, b1, w2, b2):
    with_bias = bool(np.any(np.asarray(convb)))
    if with_bias not in _CACHED_NC:
        _CACHED_NC[with_bias] = _build_program(with_bias)
    nc = _CACHED_NC[with_bias]
    globals()["_LAST_NC"] = nc
    in_maps = _prep_inputs(
        x, guidance, convw, convb, ln_g, ln_b, w1, b1, w2, b2
    )
    res = run_bass_kernel_spmd(nc, in_maps, list(range(N_CORES)))
    out = np.empty((B, C, D, H, W), dtype=np.float32)
    for core in range(N_CORES):
        b, hc = core // 2, core % 2
        y = res.results[core]["y"].reshape(NCH, D, NSLAB, HW)
        out[b, 48 * hc : 48 * hc + 48] = (
            y.transpose(2, 0, 1, 3).reshape(48, D, H, W)
        )
    return out


if __name__ == "__main__":
    rng = np.random.default_rng(0)
    ins = dict(
        x=rng.standard_normal((B, C, D, H, W), dtype=np.float32),
        guidance=rng.standard_normal((B, G), dtype=np.float32),
        convw=(rng.standard_normal((NB, C, 1, K, K, K)) * 0.1).astype(np.float32),
        convb=np.zeros((NB, C), np.float32),
        ln_g=np.ones((C + G,), np.float32),
        ln_b=np.zeros((C + G,), np.float32),
        w1=(rng.standard_normal((C + G, HID)) * 0.05).astype(np.float32),
        b1=np.zeros((HID,), np.float32),
        w2=(rng.standard_normal((HID, NB)) * 0.05).astype(np.float32),
        b2=np.zeros((NB,), np.float32),
    )
    out = kernel(**ins)
    print("kernel ran, out shape", out.shape, "mean", float(np.abs(out).mean()))


# revision 20
# speedup vs baseline: 3.7871x; 1.0056x over previous
"""AttentionGuidedDynamicRangeDWConv3D on 8 Trainium2 NeuronCores.

Module: out = sum_i softmax(MLP(LN([mean_dhw(x), guidance])))[:, i]
                * dwconv3d(x, convw[i], convb[i], dil=i+1)
Shapes: x [4,96,16,56,56] f32, 3 branches of 3x3x3 depthwise conv with
dilations 1/2/3 ('same' zero padding).

Sharding: 8 cores = (batch b in 0..3) x (channel half hc in 0..1); each
core owns 48 channels of one batch at FULL depth.

Layout trick: partitions = (channel c in 0..8) x (depth d in 0..16), so a
single bf16 matmul with a 128x128 block-banded weight matrix applies an
entire depth-band of conv taps at once: out[(c,d), hw] +=
sum_od w[c, (od,oh,ow)] * x[(c,d+od), hw + oh*56+ow].  The 81 taps
(3 branches x 27) collapse into 25 matmul passes -- one per distinct
(oh,ow) pair -- accumulated in PSUM per 448-column (8 h-row) chunk.
Depth 'same' padding falls out of band truncation (no halo).  H/W 'same'
padding is exact via trimmed 2D access patterns (bf16 matmuls allow
strided APs; fp32r would not).

Band matrices are built by the Vector engine from host-supplied
shifted-identity masks scaled by per-partition weight columns.  The gate
MLP runs redundantly per core; the global mean-pool takes one pairwise
96x6-float AllGather (15us fixed latency in the cost model).  To hide
that latency plus the MLP, slabs 0 and 1 run UNGATED: their 27
single-branch passes accumulate the three branch convs into separate
PSUM banks, Act copies them to SBUF, and once the softmax weights are
ready the Vector engine does the weighted 3-way merge.  Slabs 2-5 use
gate-folded matrices (25 passes) and a plain Act PSUM->SBUF copy.
Engine streams are in-order, so emission order is chosen to keep DVE
(matrix builds) and Act (pool reductions + PSUM copies) ahead of the
Tensor engine throughout.
"""

import sys

if "/opt/trn_rl_repo" not in sys.path:
    sys.path.insert(0, "/opt/trn_rl_repo")

import ml_dtypes
import numpy as np

import concourse.bass as bass
import concourse.mybir as mybir
import concourse.tile as tile
from concourse.bass_utils import run_bass_kernel_spmd

F32 = mybir.dt.float32
BF16 = mybir.dt.bfloat16
ALU = mybir.AluOpType
ACTF = mybir.ActivationFunctionType

B, C, D, H, W = 4, 96, 16, 56, 56
G, HID, NB = 96, 24, 3
K = 3
DILS = (1, 2, 3)
LN_EPS = 1e-5
N_CORES = 8
NCH = 8                  # channels per slab
NSLAB = 6                # slabs per core (48 channels)
NUNG = 2                 # ungated slabs (hide the collective+MLP latency)
NP = NCH * D             # 128 partitions
HW = H * W               # 3136
CHUNK = 448              # 8 h-rows; PSUM bank-sized chunk
N_CHUNKS = 7
ROWS = CHUNK // W        # 8


def _pass_list(split_branches):
    """[(oh, ow, [(od, t), ...])].  split_branches: one pass per (branch,
    (oh,ow)) with the branch's (0,0) pass first (27 passes); else one pass
    per distinct (oh,ow) with (0,0) merged across branches first (25)."""
    out = []
    for i, d in enumerate(DILS):
        for oh in (0, -d, d):
            for ow in (0, -d, d) if oh == 0 else (-d, 0, d):
                if split_branches:
                    kh, kw = oh // d + 1, ow // d + 1
                    ents = [(kd * d - d, i * 27 + kd * 9 + kh * 3 + kw)
                            for kd in range(K)]
                    out.append((oh, ow, ents))
                else:
                    if (oh, ow) == (0, 0) and i > 0:
                        continue
                    ents = []
                    for j, dj in enumerate(DILS):
                        if oh in (-dj, 0, dj) and ow in (-dj, 0, dj):
                            kh, kw = oh // dj + 1, ow // dj + 1
                            ents += [(kd * dj - dj,
                                      j * 27 + kd * 9 + kh * 3 + kw)
                                     for kd in range(K)]
                    out.append((oh, ow, ents))
    if split_branches:
        assert len(out) == 27
    else:
        assert len(out) == 25
    assert sum(len(e) for _, _, e in out) == 81
    return out


def _build_program(with_bias):
    nc = bass.Bass()
    xin = nc.dram_tensor("x", [NP, NSLAB * HW], BF16, kind="ExternalInput")
    masks_in = nc.dram_tensor("masks", [NP, 7 * NP], BF16, kind="ExternalInput")
    cwx_in = nc.dram_tensor("cwx", [NP, NSLAB * 27 * NB], F32, kind="ExternalInput")
    cbx_in = nc.dram_tensor("cbx", [NP, NSLAB * NB], F32, kind="ExternalInput")
    gdin = nc.dram_tensor("gd", [G], F32, kind="ExternalInput")
    w1t_in = nc.dram_tensor("w1t", [HID, C + G], F32, kind="ExternalInput")
    b1_in = nc.dram_tensor("b1", [HID], F32, kind="ExternalInput")
    w2_in = nc.dram_tensor("w2", [HID, NB], F32, kind="ExternalInput")
    b2_in = nc.dram_tensor("b2", [NB], F32, kind="ExternalInput")
    lng_in = nc.dram_tensor("lng", [C + G], F32, kind="ExternalInput")
    lnb_in = nc.dram_tensor("lnb", [C + G], F32, kind="ExternalInput")
    yout = nc.dram_tensor("y", [NP, NSLAB * HW], F32, kind="ExternalOutput")

    p_ung = _pass_list(True)
    p_gat = _pass_list(False)

    with tile.TileContext(nc) as tc:
        with (
            tc.tile_pool(name="sbuf", bufs=1) as pool,
            tc.tile_pool(name="mats", bufs=2) as matpool,
            tc.tile_pool(name="outs", bufs=4) as outpool,
            tc.tile_pool(name="dram", bufs=1, space="DRAM") as dpool,
            tc.tile_pool(name="psum", bufs=1, space="PSUM") as ppool,
        ):
            xs = [
                pool.tile([NP, HW], BF16, tag=f"xs{s}", name=f"xs{s}")
                for s in range(NSLAB)
            ]
            masks = pool.tile([NP, 7 * NP], BF16, tag="masks")
            cwx = pool.tile([NP, NSLAB * 27 * NB], F32, tag="cwx")
            w_exp = pool.tile([NP, NSLAB * 27 * NB], F32, tag="w_exp")
            scr = pool.tile([NP, HW], BF16, tag="scr")
            part = pool.tile([NP, NSLAB], F32, tag="part")
            grow = pool.tile([1, 2 * NP * NSLAB], F32, tag="grow")
            bb = [
                [
                    pool.tile([NP, HW], F32, tag=f"bb{s}_{b}",
                              name=f"bb{s}_{b}")
                    for b in range(NB)
                ]
                for s in range(NUNG)
            ]
            g_row = pool.tile([1, C + G], F32, tag="g_row")
            gd_row = pool.tile([1, C + G], F32, tag="gd_row")
            lng = pool.tile([1, C + G], F32, tag="lng")
            lnb = pool.tile([1, C + G], F32, tag="lnb")
            gn_row = pool.tile([1, C + G], F32, tag="gn_row")
            gn_bc = pool.tile([HID, C + G], F32, tag="gn_bc")
            w1t = pool.tile([HID, C + G], F32, tag="w1t")
            prod = pool.tile([HID, C + G], F32, tag="prod")
            hvec = pool.tile([HID, 1], F32, tag="hvec")
            b1c = pool.tile([HID, 1], F32, tag="b1c")
            w2t = pool.tile([HID, NB], F32, tag="w2t")
            l2tmp = pool.tile([HID, NB], F32, tag="l2tmp")
            z72 = pool.tile([1, HID * NB], F32, tag="z72")
            zrow = pool.tile([1, NB], F32, tag="zrow")
            b2r = pool.tile([1, NB], F32, tag="b2r")
            wts = pool.tile([1, NB], F32, tag="wts")
            wts_bc = pool.tile([NP, NB], F32, tag="wts_bc")
            s1 = pool.tile([1, 1], F32, tag="s1")
            s2 = pool.tile([1, 1], F32, tag="s2")
            s3 = pool.tile([1, 1], F32, tag="s3")
            s4 = pool.tile([1, 1], F32, tag="s4")
            if with_bias:
                cbx = pool.tile([NP, NSLAB * NB], F32, tag="cbx")
                b_exp = pool.tile([NP, NSLAB], F32, tag="b_exp")
                betmp = pool.tile([NP, NSLAB * NB], F32, tag="betmp")

            cin = dpool.tile([NP, NSLAB], F32, tag="cin")
            cout = dpool.tile([2 * NP, NSLAB], F32, tag="cout")
            gb = dpool.tile([1, C + G], F32, tag="gb")
            wb = dpool.tile([1, NB], F32, tag="wb")

            v = nc.vector
            sc = nc.scalar

            # ---- A: loads (small weights first, then x slab-by-slab) ----
            nc.sync.dma_start(out=masks[:, :], in_=masks_in[:, :])
            nc.sync.dma_start(out=cwx[:, :], in_=cwx_in[:, :])
            nc.sync.dma_start(out=xs[0][:, :], in_=xin[:, 0:HW])
            for s in range(1, NSLAB):
                nc.sync.dma_start(
                    out=xs[s][:, :], in_=xin[:, s * HW : (s + 1) * HW]
                )
            nc.sync.dma_start(out=w1t[:, :], in_=w1t_in[:, :])
            nc.sync.dma_start(out=b1c[:, :], in_=b1_in[:, None])
            nc.sync.dma_start(out=w2t[:, :], in_=w2_in[:, :])
            nc.sync.dma_start(out=b2r[:, :], in_=b2_in[None, :])
            nc.sync.dma_start(out=lng[:, :], in_=lng_in[None, :])
            nc.sync.dma_start(out=lnb[:, :], in_=lnb_in[None, :])
            nc.sync.dma_start(out=g_row[:, C:], in_=gdin[None, :])
            if with_bias:
                nc.sync.dma_start(out=cbx[:, :], in_=cbx_in[:, :])

            # ---- B: plane sums: slabs 0-3 on Act now; slab 4 threaded
            # between slab-0 branch copies; slab 5 on DVE (Act stays just
            # ahead of both the PSUM-copy demand and the collective) ----
            for s in range(NSLAB - 2):
                sc.activation(
                    scr[:, :], xs[s][:, :], ACTF.Copy,
                    accum_out=part[:, s : s + 1],
                )

            # ---- C: pairwise AllGather of raw plane sums ----
            nc.sync.dma_start(out=cin[:, :], in_=part[:, :])
            nc.gpsimd.collective_compute(
                "AllGather",
                ALU.bypass,
                replica_groups=[[2 * b, 2 * b + 1] for b in range(B)],
                ins=[cin.opt()],
                outs=[cout.opt()],
            )
            nc.sync.dma_start(out=grow[:, :], in_=cout[:, :])

            # ---- conv helpers ----
            def build_mats(s, passes, wsrc):
                mats = []
                for mi, (oh, ow, entries) in enumerate(passes):
                    mt = matpool.tile([NP, NP], BF16, tag=f"m{mi}")
                    for ei, (od, t) in enumerate(entries):
                        mk_in = masks[:, (od + 3) * NP : (od + 4) * NP]
                        wcol = wsrc[:, s * 81 + t : s * 81 + t + 1]
                        if ei == 0:
                            v.tensor_scalar(
                                out=mt[:, :], in0=mk_in, scalar1=wcol,
                                scalar2=None, op0=ALU.mult,
                            )
                        else:
                            v.scalar_tensor_tensor(
                                out=mt[:, :], in0=mk_in, scalar=wcol,
                                in1=mt[:, :], op0=ALU.mult, op1=ALU.add,
                            )
                    mats.append(mt)
                return mats

            def emit_pass(ps, pv, mt, xf, xv, ci, oh, ow, start, stop):
                if (oh, ow) == (0, 0):
                    nc.tensor.matmul(
                        ps[:, :], mt[:, :],
                        xf[:, ci * CHUNK : (ci + 1) * CHUNK],
                        start=start, stop=stop, skip_group_check=True,
                    )
                    return
                h0 = max(ci * ROWS, -oh if oh < 0 else 0)
                h1 = min(ci * ROWS + ROWS, H - (oh if oh > 0 else 0))
                if h1 <= h0:
                    return
                w0 = -ow if ow < 0 else 0
                w1 = W - (ow if ow > 0 else 0)
                nc.tensor.matmul(
                    pv[:, h0 - ci * ROWS : h1 - ci * ROWS, w0:w1],
                    mt[:, :],
                    xv[:, h0 + oh : h1 + oh, w0 + ow : w1 + ow],
                    start=start, stop=stop, skip_group_check=True,
                )

            def slab_views(s):
                xf = xs[s][:, :]
                return xf, xf.rearrange("p (h w) -> p h w", h=H, w=W)

            def emit_ungated_passmajor(s, mats, after_branch=None):
                # branch-major / pass-major: PE consumes each matrix for 7
                # chunk-matmuls (1.3us) vs its 0.5us build -- no build-pacing
                # stalls on the very first slab.  Copies fire per branch.
                xf, xv = slab_views(s)
                for b in range(NB):
                    pss = [
                        ppool.tile([NP, CHUNK], F32, tag=f"ps{(3 * b + ci) % 8}",
                                   name=f"ups{s}_{b}_{ci}")
                        for ci in range(N_CHUNKS)
                    ]
                    pvs = [ps[:, :].rearrange("p (h w) -> p h w", h=ROWS, w=W)
                           for ps in pss]
                    for k in range(9):
                        oh, ow, _ = p_ung[b * 9 + k]
                        for ci in range(N_CHUNKS):
                            emit_pass(pss[ci], pvs[ci], mats[b * 9 + k], xf, xv,
                                      ci, oh, ow, k == 0, k == 8)
                    # copy order matches the tag order the NEXT consumer
                    # (branch b+1 / the following slab) acquires, so its
                    # start=True matmuls unblock as early as possible
                    for ci in range(N_CHUNKS):
                        sc.activation(
                            bb[s][b][:, ci * CHUNK : (ci + 1) * CHUNK],
                            pss[ci][:, :], ACTF.Copy,
                        )
                    if after_branch and b in after_branch:
                        after_branch[b]()

            def emit_ungated_chunkmajor(s, mats):
                xf, xv = slab_views(s)
                for ci in range(N_CHUNKS):
                    for b in range(NB):
                        ps = ppool.tile([NP, CHUNK], F32,
                                        tag=f"ps{(3 * ci + b) % 8}",
                                        name=f"ups{s}_{b}_{ci}")
                        pv = ps[:, :].rearrange("p (h w) -> p h w", h=ROWS, w=W)
                        for k in range(9):
                            oh, ow, _ = p_ung[b * 9 + k]
                            emit_pass(ps, pv, mats[b * 9 + k], xf, xv, ci,
                                      oh, ow, k == 0, k == 8)
                        sc.activation(
                            bb[s][b][:, ci * CHUNK : (ci + 1) * CHUNK],
                            ps[:, :], ACTF.Copy,
                        )

            def emit_merge(s):
                for ci in range(N_CHUNKS):
                    sl = slice(ci * CHUNK, (ci + 1) * CHUNK)
                    ot = outpool.tile([NP, CHUNK], F32, tag=f"o{ci % 4}")
                    v.tensor_scalar(
                        out=ot[:, :], in0=bb[s][0][:, sl],
                        scalar1=wts_bc[:, 0:1], scalar2=None, op0=ALU.mult,
                    )
                    for b in (1, 2):
                        v.scalar_tensor_tensor(
                            out=ot[:, :], in0=bb[s][b][:, sl],
                            scalar=wts_bc[:, b : b + 1], in1=ot[:, :],
                            op0=ALU.mult, op1=ALU.add,
                        )
                    if with_bias:
                        v.tensor_scalar(
                            out=ot[:, :], in0=ot[:, :],
                            scalar1=b_exp[:, s : s + 1], scalar2=None,
                            op0=ALU.add,
                        )
                    nc.sync.dma_start(
                        out=yout[:, s * HW + ci * CHUNK : s * HW + (ci + 1) * CHUNK],
                        in_=ot[:, :],
                    )

            def emit_gated_out(s, ci, ps):
                ot = outpool.tile([NP, CHUNK], F32, tag=f"o{ci % 4}",
                                  name=f"ot{s}_{ci}")
                sc.activation(ot[:, :], ps[:, :], ACTF.Copy)
                if with_bias:
                    v.tensor_scalar(
                        out=ot[:, :], in0=ot[:, :],
                        scalar1=b_exp[:, s : s + 1], scalar2=None,
                        op0=ALU.add,
                    )
                nc.sync.dma_start(
                    out=yout[:, s * HW + ci * CHUNK : s * HW + (ci + 1) * CHUNK],
                    in_=ot[:, :],
                )

            def emit_gated_slab(s, passmajor=False):
                mats = build_mats(s, p_gat, w_exp)
                xf, xv = slab_views(s)
                if passmajor:
                    # consume each matrix for 7 chunk-matmuls: tolerates
                    # just-in-time builds right after the gate MLP lands
                    pss = [
                        ppool.tile([NP, CHUNK], F32, tag=f"ps{ci}",
                                   name=f"gps{s}_{ci}")
                        for ci in range(N_CHUNKS)
                    ]
                    pvs = [ps[:, :].rearrange("p (h w) -> p h w", h=ROWS, w=W)
                           for ps in pss]
                    for mi, (oh, ow, _) in enumerate(p_gat):
                        for ci in range(N_CHUNKS):
                            emit_pass(pss[ci], pvs[ci], mats[mi], xf, xv, ci,
                                      oh, ow, mi == 0, mi == len(p_gat) - 1)
                    for ci in [1, 2, 3, 4, 5, 6, 0]:
                        emit_gated_out(s, ci, pss[ci])
                    return
                for ci in range(N_CHUNKS):
                    ps = ppool.tile([NP, CHUNK], F32, tag=f"ps{ci}",
                                    name=f"gps{s}_{ci}")
                    pv = ps[:, :].rearrange("p (h w) -> p h w", h=ROWS, w=W)
                    for mi, (oh, ow, _) in enumerate(p_gat):
                        emit_pass(ps, pv, mats[mi], xf, xv, ci, oh, ow,
                                  mi == 0, mi == len(p_gat) - 1)
                    emit_gated_out(s, ci, ps)

            # ---- D: slab 0 ungated (builds + matmuls + copies) ----
            def red4():
                sc.activation(
                    scr[:, :], xs[4][:, :], ACTF.Copy,
                    accum_out=part[:, 4:5],
                )

            # p-state warmup: dummy matmuls on the masks tile while the
            # xs0 DMA is still in flight, so the real conv stream starts at
            # full clock (the tensor engine needs ~3us of continuous work)
            wps = ppool.tile([NP, CHUNK], F32, tag="ps7", name="warmps")
            for wi in range(8):
                nc.tensor.matmul(
                    wps[:, :], masks[:, 3 * NP : 4 * NP],
                    masks[:, 2 * NP : 2 * NP + CHUNK],
                    start=(wi == 0), stop=(wi == 7), skip_group_check=True,
                )

            mats0 = build_mats(0, p_ung, cwx)
            emit_ungated_chunkmajor(0, mats0)
            red4()

            # slab-5 plane sum on DVE (Act is busy; DVE has a lull here)
            v.reduce_sum(
                part[:, NSLAB - 1 : NSLAB], xs[NSLAB - 1][:, :],
                axis=mybir.AxisListType.X,
            )

            # ---- F1: slab 1 ungated ----
            mats1 = build_mats(1, p_ung, cwx)
            emit_ungated_chunkmajor(1, mats1)

            # ---- E: gate MLP ----
            # feat[48r + 8s + c] = sum_d cout[r, (c,d), s] / (D*HW)
            for r in range(2):
                gview = grow[:, r * NP * NSLAB : (r + 1) * NP * NSLAB].rearrange(
                    "a (c d s) -> a s c d", c=NCH, d=D, s=NSLAB
                )
                tview = g_row[:, 48 * r : 48 * r + 48].rearrange(
                    "a (s c) -> a s c", s=NSLAB, c=NCH
                )
                v.reduce_sum(tview, gview, axis=mybir.AxisListType.X)
            v.tensor_scalar_mul(g_row[:, 0:C], g_row[:, 0:C], 1.0 / (D * HW))

            # LayerNorm over 192 on one partition
            v.reduce_sum(s1[:, :], g_row[:, :], axis=mybir.AxisListType.X)
            v.tensor_scalar_mul(s1[:, :], s1[:, :], 1.0 / (C + G))  # mu
            v.tensor_scalar(
                out=gd_row[:, :], in0=g_row[:, :], scalar1=s1[:, :], scalar2=None,
                op0=ALU.subtract,
            )
            v.tensor_tensor(out=gn_row[:, :], in0=gd_row[:, :], in1=gd_row[:, :], op=ALU.mult)
            v.reduce_sum(s2[:, :], gn_row[:, :], axis=mybir.AxisListType.X)
            v.tensor_scalar(
                out=s2[:, :], in0=s2[:, :], scalar1=1.0 / (C + G), scalar2=LN_EPS,
                op0=ALU.mult, op1=ALU.add,
            )  # var + eps
            sc.activation(s3[:, :], s2[:, :], ACTF.Sqrt)
            # one Newton step for a clean sqrt
            v.reciprocal(s4[:, :], s3[:, :])
            v.tensor_tensor(out=s4[:, :], in0=s4[:, :], in1=s2[:, :], op=ALU.mult)
            v.tensor_tensor(out=s4[:, :], in0=s4[:, :], in1=s3[:, :], op=ALU.add)
            v.tensor_scalar_mul(s4[:, :], s4[:, :], 0.5)
            v.reciprocal(s3[:, :], s4[:, :])  # rstd
            v.tensor_scalar(
                out=gn_row[:, :], in0=gd_row[:, :], scalar1=s3[:, :], scalar2=None,
                op0=ALU.mult,
            )
            v.tensor_tensor(out=gn_row[:, :], in0=gn_row[:, :], in1=lng[:, :], op=ALU.mult)
            v.tensor_tensor(out=gn_row[:, :], in0=gn_row[:, :], in1=lnb[:, :], op=ALU.add)

            # MLP layer 1: h = gelu(gn @ w1 + b1) via row-products
            nc.sync.dma_start(out=gb[:, :], in_=gn_row[:, :])
            nc.sync.dma_start(out=gn_bc[:, :], in_=gb[:1, :].partition_broadcast(HID))
            v.tensor_tensor(out=prod[:, :], in0=w1t[:, :], in1=gn_bc[:, :], op=ALU.mult)
            v.reduce_sum(hvec[:, :], prod[:, :], axis=mybir.AxisListType.X)
            v.tensor_tensor(out=hvec[:, :], in0=hvec[:, :], in1=b1c[:, :], op=ALU.add)
            sc.activation(hvec[:, :], hvec[:, :], ACTF.Gelu)

            # MLP layer 2 via DRAM transpose bounce
            v.tensor_scalar(
                out=l2tmp[:, :], in0=w2t[:, :], scalar1=hvec[:, :], scalar2=None,
                op0=ALU.mult,
            )
            nc.sync.dma_start(out=z72[:, :], in_=l2tmp[:, :])
            z3 = z72[:, :].rearrange("a (j i) -> a j i", j=HID, i=NB)
            for i in range(NB):
                v.reduce_sum(zrow[:, i : i + 1], z3[:, :, i], axis=mybir.AxisListType.X)
            v.tensor_tensor(out=zrow[:, :], in0=zrow[:, :], in1=b2r[:, :], op=ALU.add)

            # softmax over 3
            v.reduce_max(s1[:, :], zrow[:, :], axis=mybir.AxisListType.X)
            v.tensor_scalar(
                out=zrow[:, :], in0=zrow[:, :], scalar1=s1[:, :], scalar2=None,
                op0=ALU.subtract,
            )
            sc.activation(zrow[:, :], zrow[:, :], ACTF.Exp)
            v.reduce_sum(s2[:, :], zrow[:, :], axis=mybir.AxisListType.X)
            v.reciprocal(s2[:, :], s2[:, :])
            v.tensor_scalar(
                out=wts[:, :], in0=zrow[:, :], scalar1=s2[:, :], scalar2=None,
                op0=ALU.mult,
            )

            # broadcast gate weights; fold into per-channel tap weights
            nc.sync.dma_start(out=wb[:, :], in_=wts[:, :])
            nc.sync.dma_start(out=wts_bc[:, :], in_=wb[:1, :].partition_broadcast(NP))
            for s in range(NUNG, NSLAB):
                for i in range(NB):
                    sl = slice(s * 81 + i * 27, s * 81 + (i + 1) * 27)
                    v.tensor_scalar(
                        out=w_exp[:, sl], in0=cwx[:, sl],
                        scalar1=wts_bc[:, i : i + 1], scalar2=None, op0=ALU.mult,
                    )
            if with_bias:
                for i in range(NB):
                    v.tensor_scalar(
                        out=betmp[:, i::NB], in0=cbx[:, i::NB],
                        scalar1=wts_bc[:, i : i + 1], scalar2=None, op0=ALU.mult,
                    )
                v.tensor_tensor(
                    out=b_exp[:, :], in0=betmp[:, 0::NB], in1=betmp[:, 1::NB],
                    op=ALU.add,
                )
                v.tensor_tensor(
                    out=b_exp[:, :], in0=b_exp[:, :], in1=betmp[:, 2::NB],
                    op=ALU.add,
                )

            # ---- G..J: gated slabs; merges (not PE-critical) trail ----
            emit_gated_slab(2)
            emit_merge(0)
            emit_gated_slab(3)
            emit_merge(1)
            for s in range(4, NSLAB):
                emit_gated_slab(s)

    _split_sem_waits(nc)
    return nc


_WAITSPLIT = [0]


def _split_sem_waits(nc, max_waits=1):
    """This walrus build rejects >1 SyncWait per instruction (and any wait on
    a Drain). Move excess waits onto same-engine NOPs inserted just before."""
    for bb in nc.main_func.blocks:
        insns = bb.instructions
        i = 0
        while i < len(insns):
            ins = insns[i]
            si = ins.sync_info
            limit = 0 if ins.opcode == "Drain" else max_waits
            if si is not None and si.on_wait is not None and len(si.on_wait) > limit:
                waits = list(si.on_wait)
                keep = waits[-limit:] if limit else []
                extra = waits[: len(waits) - limit]
                pos = i
                for j in range(0, len(extra), max_waits):
                    nop = mybir.InstNoOp(
                        name=f"I-waitsplit-{_WAITSPLIT[0]}", ins=[], outs=[]
                    )
                    _WAITSPLIT[0] += 1
                    nop.engine = ins.engine
                    nop.sync_info = mybir.SyncInfo(
                        on_wait=extra[j : j + max_waits], on_update=[]
                    )
                    insns.insert(pos, nop)
                    pos += 1
                    i += 1
                si.on_wait = keep
            i += 1


def _make_masks():
    m = np.zeros((NP, 7 * NP), dtype=np.float32)
    for od in range(-3, 4):
        for p in range(NP):
            q = p - od
            if q // D == p // D and 0 <= q < NP:
                m[p, (od + 3) * NP + q] = 1.0
    return m.astype(ml_dtypes.bfloat16)


def _prep_inputs(x, guidance, convw, convb, ln_g, ln_b, w1, b1, w2, b2):
    f = np.float32
    w3 = np.ascontiguousarray(convw.reshape(NB, C, 27), dtype=f)
    cb = np.ascontiguousarray(convb, dtype=f)
    common = dict(
        masks=_make_masks(),
        w1t=np.ascontiguousarray(w1.T, dtype=f),
        b1=np.ascontiguousarray(b1, dtype=f),
        w2=np.ascontiguousarray(w2, dtype=f),
        b2=np.ascontiguousarray(b2, dtype=f),
        lng=np.ascontiguousarray(ln_g, dtype=f),
        lnb=np.ascontiguousarray(ln_b, dtype=f),
    )
    in_maps = []
    for core in range(N_CORES):
        b, hc = core // 2, core % 2
        ch0 = 48 * hc
        # xs[p=c*16+d, s*HW+j] = x[b, ch0+8s+c, d, j]
        arr = np.ascontiguousarray(x[b, ch0 : ch0 + 48], dtype=f)
        arr = arr.reshape(NSLAB, NCH, D, HW).transpose(1, 2, 0, 3).reshape(
            NP, NSLAB * HW
        )
        # cwx[p=c*16+d, s*81+t] = convw[br, ch0+8s+c, t27]  (d-independent)
        cw = w3[:, ch0 : ch0 + 48, :].reshape(NB, NSLAB, NCH, 27)
        cw = cw.transpose(2, 1, 0, 3).reshape(NCH, NSLAB * NB * 27)
        cwx = np.repeat(cw, D, axis=0)  # row c*16+d <- cw[c]
        cbs = cb[:, ch0 : ch0 + 48].reshape(NB, NSLAB, NCH)
        cbs = cbs.transpose(2, 1, 0).reshape(NCH, NSLAB * NB)
        cbx = np.repeat(cbs, D, axis=0)
        in_maps.append(
            dict(
                x=arr.astype(ml_dtypes.bfloat16),
                gd=np.ascontiguousarray(guidance[b], dtype=f),
                cwx=np.ascontiguousarray(cwx, dtype=f),
                cbx=np.ascontiguousarray(cbx, dtype=f),
                **common,
            )
        )
    return in_maps


_CACHED_NC = {}


def kernel(x, guidance, convw, convb, ln_g, ln_b, w1, b1, w2, b2):
    with_bias = bool(np.any(np.asarray(convb)))
    if with_bias not in _CACHED_NC:
        _CACHED_NC[with_bias] = _build_program(with_bias)
    nc = _CACHED_NC[with_bias]
    globals()["_LAST_NC"] = nc
    in_maps = _prep_inputs(
        x, guidance, convw, convb, ln_g, ln_b, w1, b1, w2, b2
    )
    res = run_bass_kernel_spmd(nc, in_maps, list(range(N_CORES)))
    out = np.empty((B, C, D, H, W), dtype=np.float32)
    for core in range(N_CORES):
        b, hc = core // 2, core % 2
        y = res.results[core]["y"].reshape(NCH, D, NSLAB, HW)
        out[b, 48 * hc : 48 * hc + 48] = (
            y.transpose(2, 0, 1, 3).reshape(48, D, H, W)
        )
    return out


if __name__ == "__main__":
    rng = np.random.default_rng(0)
    ins = dict(
        x=rng.standard_normal((B, C, D, H, W), dtype=np.float32),
        guidance=rng.standard_normal((B, G), dtype=np.float32),
        convw=(rng.standard_normal((NB, C, 1, K, K, K)) * 0.1).astype(np.float32),
        convb=np.zeros((NB, C), np.float32),
        ln_g=np.ones((C + G,), np.float32),
        ln_b=np.zeros((C + G,), np.float32),
        w1=(rng.standard_normal((C + G, HID)) * 0.05).astype(np.float32),
        b1=np.zeros((HID,), np.float32),
        w2=(rng.standard_normal((HID, NB)) * 0.05).astype(np.float32),
        b2=np.zeros((NB,), np.float32),
    )
    out = kernel(**ins)
    print("kernel ran, out shape", out.shape, "mean", float(np.abs(out).mean()))
